# revision 12
# baseline (speedup 1.0000x reference)
"""Trainium2 Bass kernel for a dense transformer encoder layer.

Shapes (hardcoded): B=2, L=2048, D=1024, F=4096, H=16 heads, dk=64.
Sharding over 8 NeuronCores: core c handles batch b=c//4 and query-row
quarter r=c%4 (512 rows). K/V projections for the full batch are
computed per core (replicated within the 4-core batch group; collectives
measured too slow here due to cross-core launch skew). The kernel fuses
projections INTO the attention loop: the softmax exp stream (scalar
engine, ~96us) is the long pole of attention, and K/V projection
matmuls are emitted interleaved with the score/context matmuls so the
tensor engine stays dense while exp runs. Rescale is fused per
head-pair; the epilogue (w_o + LN1 + FFN + LN2) is pipelined per query
tile. All matmuls run in bf16 with fp32 PSUM accumulation.
"""
import os
import sys
import types

sys.path.insert(0, "/opt/trn_rl_repo")

import numpy as np
import ml_dtypes

import concourse.bass as bass
import concourse.tile as tile
import concourse.mybir as mybir
from contextlib import ExitStack

f32 = mybir.dt.float32
bf16 = mybir.dt.bfloat16
AF = mybir.ActivationFunctionType
ALU = mybir.AluOpType

B, L, D, F, H, DK = 2, 2048, 1024, 4096, 16, 64
RQ = 512          # query rows per core
NCORES = 8
EPS = 1e-6
KT = D // 128      # 8 contraction tiles over D
NL = L // 512      # 4 column chunks over L
LT = L // 128      # 16 key tiles
FT = F // 128      # 32 f-tiles
VSTR = 65          # per-head stride in v tiles (64 v cols + ones)

_PATCHED = False


def _install_patches():
    """Register the NTFF profile hook (if available) and wrap the BIR
    compile step to split multi-wait instructions (this walrus build
    accepts at most one sync-wait per instruction)."""
    global _PATCHED
    if _PATCHED:
        return
    _PATCHED = True

    if "antenv.axon_hooks" not in sys.modules:
        try:
            from trn_agent_boot.trn_boot import _ntff_profile_via_ctypes
            hook = _ntff_profile_via_ctypes("/opt/axon/libaxon_pjrt.so")
        except Exception:
            hook = None
        mod = types.ModuleType("antenv.axon_hooks")
        mod.get_axon_ntff_profile_hook = lambda: hook
        mod.set_axon_ntff_profile_hook = lambda h: None
        sys.modules["antenv.axon_hooks"] = mod

    import json

    def _split_multiwaits(bir_bytes):
        d = json.loads(bir_bytes)
        ctr = 0
        for fn in d.get("functions", []):
            for blk in fn.get("blocks", []):
                out = []
                for inst in blk.get("instructions", []):
                    si = inst.get("sync_info")
                    ow = (si or {}).get("on_wait") or []
                    if len(ow) > 1 and inst.get("engine", "Unassigned") != "Unassigned":
                        for w in ow[:-1]:
                            out.append({
                                "debug": inst.get("debug", 0),
                                "engine": inst["engine"],
                                "ins": [], "outs": [],
                                "name": f"I-antsw{ctr}",
                                "opcode": "NoOp",
                                "sync_info": {"on_update": [], "on_wait": [w]},
                            })
                            ctr += 1
                        si["on_wait"] = [ow[-1]]
                    out.append(inst)
                blk["instructions"] = out
        return json.dumps(d).encode()

    import concourse.bass_utils as bu
    import concourse.bass2jax as b2j

    orig = bu.compile_bir_kernel

    def patched(bir_json, tmpdir, neff_name="file.neff"):
        return orig(_split_multiwaits(bir_json), tmpdir, neff_name=neff_name)

    bu.compile_bir_kernel = patched
    b2j.compile_bir_kernel = patched


def _build_program(flags):
    nc = bass.Bass("TRN2", target_bir_lowering=False, debug=False,
                   num_devices=NCORES)

    def din(name, shape, dt):
        return nc.dram_tensor(name, shape, dt, kind="ExternalInput").ap()

    xT = din("xT", [D, L], bf16)            # batch x, transposed
    xTq = din("xTq", [D, RQ], bf16)         # this core's columns of x[b].T
    xr = din("xr", [RQ, D], f32)            # this core's rows (residual)
    cosr = din("cosr", [128, L], bf16)      # cos table, [p%32] replicated
    sinr = din("sinr", [128, L], bf16)      # sign-baked sin table
    qcos = din("qcos", [128, RQ], bf16)     # cos table slice for these rows
    qsin = din("qsin", [128, RQ], bf16)     # sign-baked sin slice
    wq = din("wq", [D, D], bf16)
    wk = din("wk", [D, D], bf16)
    wv = din("wv", [D, D], bf16)
    wo = din("wo", [D, D], bf16)
    w1 = din("w1", [D, F], bf16)
    w2 = din("w2", [F, D], bf16)
    b1t = din("b1t", [128, F // 128], f32)  # b1 reshaped per-partition
    ident = din("ident", [128, 128], f32)
    onehot = din("onehot", [2, 128], bf16)
    bo = din("bo", [1, D], f32)
    b2r = din("b2r", [1, D], f32)
    g1 = din("g1", [1, D], f32)
    be1 = din("be1", [1, D], f32)
    g2 = din("g2", [1, D], f32)
    be2 = din("be2", [1, D], f32)
    y = nc.dram_tensor("y", [RQ, D], f32, kind="ExternalOutput").ap()

    def bcast_ap(ap2d, width):
        return bass.AP(tensor=ap2d.tensor, offset=ap2d.offset,
                       ap=[[0, 128], [1, width]])

    with tile.TileContext(nc) as tc:
      with ExitStack() as top:
        consts = top.enter_context(tc.tile_pool(name="consts", bufs=1))
        poolP1 = top.enter_context(tc.tile_pool(name="p1", bufs=1))
        dramp = top.enter_context(tc.tile_pool(name="dramp", bufs=1,
                                               space="DRAM"))
        h_dram = dramp.tile([RQ, D], f32, tag="h_dram", name="h_dram")
        stackCtx = ExitStack()
        poolCtx = stackCtx.enter_context(tc.tile_pool(name="pctx", bufs=1))
        stackP2 = ExitStack()
        poolP2 = stackP2.enter_context(tc.tile_pool(name="p2", bufs=1))
        poolE = stackP2.enter_context(tc.tile_pool(name="pe", bufs=3))
        poolCR = stackP2.enter_context(tc.tile_pool(name="pcr", bufs=1))
        stackP3 = ExitStack()
        poolP3 = stackP3.enter_context(tc.tile_pool(name="p3", bufs=1))
        poolRW = stackP3.enter_context(tc.tile_pool(name="prw", bufs=2))

        _qs = [nc.sync, nc.scalar, nc.gpsimd]
        _qi = [0]

        def ld(dst, src):
            # preamble loads: round-robin all three DMA-capable queues
            _qs[_qi[0] % 3].dma_start(dst, src)
            _qi[0] += 1

        _qs2 = [nc.sync, nc.gpsimd]
        _qi2 = [0]

        def ld2(dst, src):
            # loads issued while the exp stream owns the scalar queue
            _qs2[_qi2[0] % 2].dma_start(dst, src)
            _qi2[0] += 1

        # --- long-lived constants ---
        ident_sb = consts.tile([128, 128], f32, tag="ident", name="ident")
        nc.sync.dma_start(ident_sb[:], ident[:])
        b1_sb = consts.tile([128, F // 128], f32, tag="b1", name="b1")
        nc.scalar.dma_start(b1_sb[:], b1t[:])
        onehot_sb = consts.tile([2, 128], bf16, tag="onehot", name="onehot")
        nc.gpsimd.dma_start(onehot_sb[:], onehot[:])
        eps_sb = consts.tile([128, 1], f32, tag="eps", name="eps")
        nc.vector.memset(eps_sb[:], EPS)

        def rep_const(ap2d, use, tag):
            if not use:
                return None
            t = consts.tile([128, D], f32, tag=tag, name=tag)
            nc.gpsimd.dma_start(out=t[:], in_=bcast_ap(ap2d, D))
            return t

        bo_rep = rep_const(bo, flags["use_bo"], "bo")
        b2_rep = rep_const(b2r, flags["use_b2"], "b2")
        g1_rep = rep_const(g1, flags["use_g1"], "g1")
        be1_rep = rep_const(be1, flags["use_be1"], "be1")
        g2_rep = rep_const(g2, flags["use_g2"], "g2")
        be2_rep = rep_const(be2, flags["use_be2"], "be2")

        # hT persists from the w_o phase into the FFN (top-level pool)
        hT = [poolP1.tile([128, RQ], bf16, tag=f"hT{k}", name=f"hT{k}")
              for k in range(KT)]
        # ctxT persists into the w_o phase only
        ctxT = [poolCtx.tile([128, RQ], bf16, tag=f"ctxT{m}", name=f"ctxT{m}")
                for m in range(KT)]

        # attention-lived arrays
        kTr = [poolP2.tile([128, L], bf16, tag=f"kTr{m}", name=f"kTr{m}")
               for m in range(KT)]
        qTr = [poolP2.tile([128, RQ], bf16, tag=f"qTr{m}", name=f"qTr{m}")
               for m in range(KT)]
        v_lo = [poolP2.tile([128, 8 * VSTR], bf16, tag=f"vl{t}",
                            name=f"vl{t}") for t in range(LT)]
        v_hi = [poolP2.tile([128, 8 * VSTR], bf16, tag=f"vh{t}",
                            name=f"vh{t}") for t in range(LT)]
        ctxraw = [poolP2.tile([VSTR, RQ], bf16, tag=f"cr{h}", name=f"cr{h}")
                  for h in range(H)]

        # projection inputs
        xT_sb = [poolP3.tile([128, L], bf16, tag=f"xT{k}", name=f"xT{k}")
                 for k in range(KT)]
        xTq_sb = [poolP3.tile([128, RQ], bf16, tag=f"xTq{k}", name=f"xTq{k}")
                  for k in range(KT)]
        wk_sb = [poolP3.tile([128, D], bf16, tag=f"wk{k}", name=f"wk{k}")
                 for k in range(KT)]
        cos_sb = poolP3.tile([128, L], bf16, tag="cos", name="cos")
        sin_sb = poolP3.tile([128, L], bf16, tag="sin", name="sin")
        qcos_sb = poolP3.tile([128, RQ], bf16, tag="qcos", name="qcos")
        qsin_sb = poolP3.tile([128, RQ], bf16, tag="qsin", name="qsin")
        stackP3b = ExitStack()
        poolWQ = stackP3b.enter_context(tc.tile_pool(name="pwq", bufs=1))
        wq_sb = [poolWQ.tile([128, D], bf16, tag=f"wq{k}", name=f"wq{k}")
                 for k in range(KT)]

        # load order: q-proj inputs first, then k, then v
        for k in range(KT):
            ld(wq_sb[k][:], wq[k * 128:(k + 1) * 128, :])
            ld(xTq_sb[k][:], xTq[k * 128:(k + 1) * 128, :])
        ld(qcos_sb[:], qcos[:])
        ld(qsin_sb[:], qsin[:])
        for k in range(KT):
            ld(wk_sb[k][:], wk[k * 128:(k + 1) * 128, :])
            ld(xT_sb[k][:], xT[k * 128:(k + 1) * 128, :])
        ld(cos_sb[:], cosr[:])
        ld(sin_sb[:], sinr[:])

        def rope_chunk(ps, cos_sl, sin_sl, dst):
            """dst = ps*cos + swap32(ps*sin) (sign-baked sin table)."""
            tct = poolRW.tile([128, 512], bf16, tag="rtc", name="rtc")
            nc.vector.tensor_mul(tct[:], ps, cos_sl)
            tsn = poolRW.tile([128, 512], bf16, tag="rtm", name="rtm")
            nc.vector.tensor_mul(tsn[:], ps, sin_sl)
            tsw = poolRW.tile([128, 512], bf16, tag="tsw", name="tsw")
            for g in range(2):
                o = g * 64
                nc.sync.dma_start(tsw[o:o + 32, :], tsn[o + 32:o + 64, :])
                nc.gpsimd.dma_start(tsw[o + 32:o + 64, :], tsn[o:o + 32, :])
            nc.vector.tensor_add(dst, tct[:], tsw[:])

        with tc.tile_pool(name="pjps", bufs=2, space="PSUM") as pjps, \
             tc.tile_pool(name="scps", bufs=2, space="PSUM") as scps, \
             tc.tile_pool(name="ctxps", bufs=1, space="PSUM") as ctxps:

            def q_proj(m):
                msl = slice(m * 128, m * 128 + 128)
                ps = pjps.tile([128, 512], f32, tag="pj", name="pj")
                for k in range(KT):
                    nc.tensor.matmul(ps[:], wq_sb[k][:, msl], xTq_sb[k][:],
                                     start=(k == 0), stop=(k == KT - 1))
                rope_chunk(ps[:], qcos_sb[:], qsin_sb[:], qTr[m][:])

            def k_proj(m, n):
                msl = slice(m * 128, m * 128 + 128)
                nsl = slice(n * 512, n * 512 + 512)
                ps = pjps.tile([128, 512], f32, tag="pj", name="pj")
                for k in range(KT):
                    nc.tensor.matmul(ps[:], wk_sb[k][:, msl],
                                     xT_sb[k][:, nsl],
                                     start=(k == 0), stop=(k == KT - 1))
                rope_chunk(ps[:], cos_sb[:, nsl], sin_sb[:, nsl],
                           kTr[m][:, nsl])

            # q projection first; its weights then make room for wv
            for m in range(KT):
                q_proj(m)
            stackP3b.close()
            poolWV = stackP3.enter_context(
                tc.tile_pool(name="pwv", bufs=1))
            wv_sb = [poolWV.tile([128, D], bf16, tag=f"wv{k}",
                                 name=f"wv{k}") for k in range(KT)]
            for k in range(KT):
                ld(wv_sb[k][:], wv[k * 128:(k + 1) * 128, :])

            def v_proj(half, t):
                tsl = slice(t * 128, t * 128 + 128)
                vt = v_lo[t] if half == 0 else v_hi[t]
                ps = pjps.tile([128, 512], f32, tag="pj", name="pj")
                for k in range(KT):
                    nc.tensor.matmul(ps[:], xT_sb[k][:, tsl],
                                     wv_sb[k][:, half * 512:half * 512 + 512],
                                     start=(k == 0), stop=(k == KT - 1))
                vt2 = v_lo[t] if half == 0 else v_hi[t]
                vview = vt2[:].rearrange("p (h e) -> p h e", h=8)
                ps_view = ps[:].rearrange("p (h e) -> p h e", h=8)
                nc.vector.tensor_copy(vview[:, :, 0:DK], ps_view[:])
                nc.vector.memset(vview[:, :, DK:DK + 1], 1.0)

            # deferred projection work, emitted interleaved into attention
            stream = []
            for t in range(LT):
                stream.append(("v0", 0, t))
            for m in range(1, 5):
                for n in range(NL):
                    stream.append(("k", m, n))
            for t in range(LT):
                stream.append(("v1", 1, t))
            for m in range(5, KT):
                for n in range(NL):
                    stream.append(("k", m, n))
            v_pos = {}   # (half, t) -> stream index
            k_pos = {}   # m -> index of last chunk of k m
            for i, it in enumerate(stream):
                if it[0] == "k":
                    k_pos[it[1]] = i
                else:
                    v_pos[(it[1], it[2])] = i
            fed = [0]

            def emit(it):
                if it[0] == "k":
                    k_proj(it[1], it[2])
                else:
                    v_proj(it[1], it[2])

            def feed(n):
                for _ in range(n):
                    if fed[0] < len(stream):
                        emit(stream[fed[0]])
                        fed[0] += 1

            def drain_to(idx):
                while fed[0] <= idx:
                    emit(stream[fed[0]])
                    fed[0] += 1

            # preamble: kTr[0]
            for n in range(NL):
                k_proj(0, n)

            # ---- attention (projection stream fed between steps) ----
            for hp in range(KT):
                if hp >= 1:
                    drain_to(k_pos[hp])
                hA, hB = 2 * hp, 2 * hp + 1
                half = 0 if hp < 4 else 1
                vt = v_lo if hp < 4 else v_hi
                ca = (hA % 8) * VSTR
                cb = (hB % 8) * VSTR
                cpsA = ctxps.tile([VSTR, RQ], f32, tag="cpsA", name="cpsA")
                cpsB = ctxps.tile([VSTR, RQ], f32, tag="cpsB", name="cpsB")
                pend = None
                for kt in range(LT):
                    off = kt * 128
                    sc = scps.tile([128, 2 * RQ], f32, tag="sc", name="sc")
                    nc.tensor.matmul(sc[:, 0:RQ],
                                     kTr[hp][0:64, off:off + 128],
                                     qTr[hp][0:64, :], start=True, stop=True)
                    nc.tensor.matmul(sc[:, RQ:2 * RQ],
                                     kTr[hp][64:128, off:off + 128],
                                     qTr[hp][64:128, :], start=True, stop=True)
                    e = poolE.tile([128, 2 * RQ], bf16, tag="e", name="e")
                    nc.scalar.activation(e[:], sc[:], AF.Exp, scale=0.125)
                    feed(1)
                    if pend is not None:
                        ep, ktp = pend
                        nc.tensor.matmul(cpsA[:], vt[ktp][:, ca:ca + VSTR],
                                         ep[:, 0:RQ],
                                         start=(ktp == 0), stop=False)
                        nc.tensor.matmul(cpsB[:], vt[ktp][:, cb:cb + VSTR],
                                         ep[:, RQ:2 * RQ],
                                         start=(ktp == 0), stop=False)
                    drain_to(v_pos[(half, min(kt + 1, LT - 1))])
                    pend = (e, kt)
                ep, ktp = pend
                nc.tensor.matmul(cpsA[:], vt[ktp][:, ca:ca + VSTR],
                                 ep[:, 0:RQ], start=False, stop=True)
                nc.tensor.matmul(cpsB[:], vt[ktp][:, cb:cb + VSTR],
                                 ep[:, RQ:2 * RQ], start=False, stop=True)
                nc.vector.tensor_copy(ctxraw[hA][:], cpsA[:])
                nc.vector.tensor_copy(ctxraw[hB][:], cpsB[:])
                s2 = poolCR.tile([2, RQ], bf16, tag="s2", name="s2")
                nc.sync.dma_start(s2[0:1, :], ctxraw[hA][64:65, :])
                nc.sync.dma_start(s2[1:2, :], ctxraw[hB][64:65, :])
                rec2 = poolCR.tile([2, RQ], f32, tag="rc", name="rc")
                nc.vector.reciprocal(rec2[:], s2[:])
                rcb = poolCR.tile([2, RQ], bf16, tag="rcb", name="rcb")
                nc.vector.tensor_copy(rcb[:], rec2[:])
                for h, cr in ((hA, ctxraw[hA]), (hB, ctxraw[hB])):
                    half2 = h % 2
                    rp = pjps.tile([128, 512], f32, tag="pj", name="pj")
                    nc.tensor.matmul(
                        rp[0:64, :],
                        onehot_sb[0:2, half2 * 64:half2 * 64 + 64],
                        rcb[:], start=True, stop=True)
                    dst = ctxT[hp][half2 * 64:half2 * 64 + 64, :]
                    nc.vector.tensor_mul(dst, cr[0:64, :], rp[0:64, :])

            stackP3.close()

        stackP2.close()

        # layer norm helper (takes its workspace pool)
        def layer_norm(dst, src, g_rep, be_rep, wpool):
            sview = src.rearrange("p (s d) -> p s d", s=2)
            stats = wpool.tile([128, 2, 6], f32, tag="lnstats",
                               name="lnstats")
            for sg in range(2):
                nc.vector.bn_stats(stats[:, sg, :], sview[:, sg, :])
            mv = wpool.tile([128, 2], f32, tag="lnmv", name="lnmv")
            nc.vector.bn_aggr(mv[:], stats[:])
            std = wpool.tile([128, 1], f32, tag="lnstd", name="lnstd")
            nc.scalar.activation(std[:], mv[:, 1:2], AF.Sqrt, bias=eps_sb[:])
            rstd = wpool.tile([128, 1], f32, tag="lnrstd", name="lnrstd")
            nc.vector.reciprocal(rstd[:], std[:])
            nc.vector.tensor_scalar(dst, src, mv[:, 0:1], rstd[:],
                                    op0=ALU.subtract, op1=ALU.mult)
            if g_rep is not None:
                nc.vector.tensor_mul(dst, dst, g_rep[:])
            if be_rep is not None:
                nc.vector.tensor_add(dst, dst, be_rep[:])

        # ---- w_o + residual + LN1 + transpose (per query tile) ----
        with tc.tile_pool(name="pwo", bufs=1) as poolWO, \
             tc.tile_pool(name="ph3w", bufs=2) as ph3w, \
             tc.tile_pool(name="aops", bufs=2, space="PSUM") as aops, \
             tc.tile_pool(name="tpps", bufs=4, space="PSUM") as tpps:
            wo_sb = [poolWO.tile([128, D], bf16, tag=f"wo{k}", name=f"wo{k}")
                     for k in range(KT)]
            xr_sb = [poolWO.tile([128, D], f32, tag=f"xr{t}", name=f"xr{t}")
                     for t in range(4)]
            for k in range(KT):
                ld2(wo_sb[k][:], wo[k * 128:(k + 1) * 128, :])
            for t in range(4):
                ld2(xr_sb[t][:], xr[t * 128:(t + 1) * 128, :])
            for qt in range(4):
                qsl = slice(qt * 128, qt * 128 + 128)
                ps = aops.tile([128, D], f32, tag="ao", name="ao")
                for half in range(2):
                    osl = slice(half * 512, half * 512 + 512)
                    for m in range(KT):
                        nc.tensor.matmul(ps[:, osl], ctxT[m][:, qsl],
                                         wo_sb[m][:, osl],
                                         start=(m == 0), stop=(m == KT - 1))
                res = ph3w.tile([128, D], f32, tag="res", name="res")
                nc.vector.tensor_add(res[:], ps[:], xr_sb[qt][:])
                if bo_rep is not None:
                    nc.vector.tensor_add(res[:], res[:], bo_rep[:])
                hq = ph3w.tile([128, D], f32, tag="hq", name="hq")
                layer_norm(hq[:], res[:], g1_rep, be1_rep, ph3w)
                nc.sync.dma_start(h_dram[qsl, :], hq[:])
                for m in range(KT):
                    tp = tpps.tile([128, 128], f32, tag="tp", name="tp")
                    nc.tensor.transpose(tp[:], hq[:, m * 128:m * 128 + 128],
                                        ident_sb[:])
                    nc.vector.tensor_copy(hT[m][:, qsl], tp[:])

        stackCtx.close()

        # ---- FFN (weights into all the freed space) ----
        with tc.tile_pool(name="pffn", bufs=1) as pffn, \
             tc.tile_pool(name="ph4w", bufs=2) as ph4w:
            ff1rT = [pffn.tile([128, RQ], bf16, tag=f"ff1{t}",
                               name=f"ff1{t}") for t in range(FT)]
            w1_sb = [pffn.tile([128, F], bf16, tag=f"w1{k}", name=f"w1{k}")
                     for k in range(KT)]
            w2_sb = [pffn.tile([128, D], bf16, tag=f"w2{k}", name=f"w2{k}")
                     for k in range(FT)]
            # w1 column-block-major so ffn1 chain ft=0 starts early
            for g in range(8):
                gsl = slice(g * 512, (g + 1) * 512)
                for k in range(KT):
                    ld2(w1_sb[k][:, gsl], w1[k * 128:(k + 1) * 128, gsl])
            # w2 half-column-major
            for half in range(2):
                osl = slice(half * 512, half * 512 + 512)
                for k in range(FT):
                    ld2(w2_sb[k][:, osl], w2[k * 128:(k + 1) * 128, osl])

            # ---- FFN up + ReLU ----
            with tc.tile_pool(name="f1ps", bufs=4, space="PSUM") as f1ps:
                for ft in range(FT):
                    fsl = slice(ft * 128, ft * 128 + 128)
                    ps = f1ps.tile([128, RQ], f32, tag="f1", name="f1")
                    for k in range(KT):
                        nc.tensor.matmul(ps[:], w1_sb[k][:, fsl], hT[k][:],
                                         start=(k == 0), stop=(k == KT - 1))
                    nc.scalar.activation(ff1rT[ft][:], ps[:], AF.Relu,
                                         bias=b1_sb[:, ft:ft + 1])

            # ---- FFN down + LN2 ----
            with tc.tile_pool(name="f2ps", bufs=3, space="PSUM") as f2ps:
                for qt in range(4):
                    qsl = slice(qt * 128, qt * 128 + 128)
                    ps = f2ps.tile([128, D], f32, tag="f2", name="f2")
                    for half in range(2):
                        osl = slice(half * 512, half * 512 + 512)
                        for ft in range(FT):
                            nc.tensor.matmul(ps[:, osl], ff1rT[ft][:, qsl],
                                             w2_sb[ft][:, osl],
                                             start=(ft == 0),
                                             stop=(ft == FT - 1))
                    hback = ph4w.tile([128, D], f32, tag="hback",
                                      name="hback")
                    nc.sync.dma_start(hback[:], h_dram[qsl, :])
                    res = ph4w.tile([128, D], f32, tag="res2", name="res2")
                    nc.vector.tensor_add(res[:], ps[:], hback[:])
                    if b2_rep is not None:
                        nc.vector.tensor_add(res[:], res[:], b2_rep[:])
                    o = ph4w.tile([128, D], f32, tag="out", name="out")
                    layer_norm(o[:], res[:], g2_rep, be2_rep, ph4w)
                    nc.sync.dma_start(y[qt * 128:(qt + 1) * 128, :], o[:])

    return nc


_CACHED = {}


def _get_program(flags):
    key = tuple(sorted(flags.items()))
    if key not in _CACHED:
        _CACHED[key] = _build_program(flags)
    return _CACHED[key]


def kernel(x, w_q, w_k, w_v, w_o, b_o, gamma1, beta1, gamma2, beta2,
           w1, b1, w2, b2, _trace=False):
    _install_patches()
    from concourse import bass_utils

    bf = ml_dtypes.bfloat16
    x = np.asarray(x, np.float32)
    flags = {
        "use_bo": not np.all(np.asarray(b_o) == 0),
        "use_b2": not np.all(np.asarray(b2) == 0),
        "use_g1": not np.all(np.asarray(gamma1) == 1),
        "use_be1": not np.all(np.asarray(beta1) == 0),
        "use_g2": not np.all(np.asarray(gamma2) == 1),
        "use_be2": not np.all(np.asarray(beta2) == 0),
    }
    nc = _get_program(flags)

    # host-side shared prep
    inv_freq = (1.0 / (10000.0 ** (np.arange(0, DK, 2, dtype=np.float64) / DK)))
    freqs = np.arange(L, dtype=np.float64)[:, None] * inv_freq      # [L, 32]
    cos = np.cos(freqs).T.astype(np.float32)                        # [32, L]
    sin = np.sin(freqs).T.astype(np.float32)
    cos_rep = np.tile(cos, (4, 1)).astype(bf)                       # [128, L]
    sin_sign = np.concatenate([sin, -sin, sin, -sin], 0).astype(bf)

    common = {
        "cosr": cos_rep, "sinr": sin_sign,
        "wq": w_q.astype(bf), "wk": w_k.astype(bf), "wv": w_v.astype(bf),
        "wo": w_o.astype(bf), "w1": w1.astype(bf), "w2": w2.astype(bf),
        "b1t": np.ascontiguousarray(
            np.asarray(b1, np.float32).reshape(F // 128, 128).T),
        "ident": np.eye(128, dtype=np.float32),
        "onehot": np.concatenate(
            [np.concatenate([np.ones((1, 64), np.float32),
                             np.zeros((1, 64), np.float32)], 1),
             np.concatenate([np.zeros((1, 64), np.float32),
                             np.ones((1, 64), np.float32)], 1)],
            0).astype(bf),
        "bo": np.asarray(b_o, np.float32).reshape(1, D),
        "b2r": np.asarray(b2, np.float32).reshape(1, D),
        "g1": np.asarray(gamma1, np.float32).reshape(1, D),
        "be1": np.asarray(beta1, np.float32).reshape(1, D),
        "g2": np.asarray(gamma2, np.float32).reshape(1, D),
        "be2": np.asarray(beta2, np.float32).reshape(1, D),
    }
    xT_all = [np.ascontiguousarray(x[b].T).astype(bf) for b in range(B)]

    in_maps = []
    for c in range(NCORES):
        b, r = c // 4, c % 4
        rows = slice(r * RQ, (r + 1) * RQ)
        m = dict(common)
        m["xT"] = xT_all[b]
        m["xTq"] = np.ascontiguousarray(xT_all[b][:, rows])
        m["xr"] = np.ascontiguousarray(x[b, rows, :])
        m["qcos"] = np.ascontiguousarray(cos_rep[:, rows])
        m["qsin"] = np.ascontiguousarray(sin_sign[:, rows])
        in_maps.append(m)

    res = bass_utils.run_bass_kernel_spmd(
        nc, in_maps, core_ids=list(range(NCORES)), trace=_trace)

    out = np.empty((B, L, D), np.float32)
    for c in range(NCORES):
        b, r = c // 4, c % 4
        out[b, r * RQ:(r + 1) * RQ, :] = res.results[c]["y"]
    if _trace:
        kernel.last_exec_time_ns = res.exec_time_ns
    return out


# revision 14
# speedup vs baseline: 1.0450x; 1.0450x over previous
"""Trainium2 Bass kernel for a dense transformer encoder layer.

Shapes (hardcoded): B=2, L=2048, D=1024, F=4096, H=16 heads, dk=64.
Sharding over 8 NeuronCores: core c handles batch b=c//4 and query-row
quarter r=c%4 (512 rows). K/V projections for the full batch are
computed per core (replicated within the 4-core batch group; collectives
measured too slow here due to cross-core launch skew). The kernel fuses
projections INTO the attention loop: the softmax exp stream (scalar
engine, ~96us) is the long pole of attention, and K/V projection
matmuls are emitted interleaved with the score/context matmuls so the
tensor engine stays dense while exp runs. Rescale is fused per
head-pair; the epilogue (w_o + LN1 + FFN + LN2) is pipelined per query
tile. All matmuls run in bf16 with fp32 PSUM accumulation.
"""
import os
import sys
import types

sys.path.insert(0, "/opt/trn_rl_repo")

import numpy as np
import ml_dtypes

import concourse.bass as bass
import concourse.tile as tile
import concourse.mybir as mybir
from contextlib import ExitStack

f32 = mybir.dt.float32
bf16 = mybir.dt.bfloat16
AF = mybir.ActivationFunctionType
ALU = mybir.AluOpType

B, L, D, F, H, DK = 2, 2048, 1024, 4096, 16, 64
RQ = 512          # query rows per core
NCORES = 8
EPS = 1e-6
KT = D // 128      # 8 contraction tiles over D
NL = L // 512      # 4 column chunks over L
LT = L // 128      # 16 key tiles
FT = F // 128      # 32 f-tiles
VSTR = 65          # per-head stride in v tiles (64 v cols + ones)

_PATCHED = False


def _install_patches():
    """Register the NTFF profile hook (if available) and wrap the BIR
    compile step to split multi-wait instructions (this walrus build
    accepts at most one sync-wait per instruction)."""
    global _PATCHED
    if _PATCHED:
        return
    _PATCHED = True

    if "antenv.axon_hooks" not in sys.modules:
        try:
            from trn_agent_boot.trn_boot import _ntff_profile_via_ctypes
            hook = _ntff_profile_via_ctypes("/opt/axon/libaxon_pjrt.so")
        except Exception:
            hook = None
        mod = types.ModuleType("antenv.axon_hooks")
        mod.get_axon_ntff_profile_hook = lambda: hook
        mod.set_axon_ntff_profile_hook = lambda h: None
        sys.modules["antenv.axon_hooks"] = mod

    import json

    def _split_multiwaits(bir_bytes):
        d = json.loads(bir_bytes)
        ctr = 0
        for fn in d.get("functions", []):
            for blk in fn.get("blocks", []):
                out = []
                for inst in blk.get("instructions", []):
                    si = inst.get("sync_info")
                    ow = (si or {}).get("on_wait") or []
                    if len(ow) > 1 and inst.get("engine", "Unassigned") != "Unassigned":
                        for w in ow[:-1]:
                            out.append({
                                "debug": inst.get("debug", 0),
                                "engine": inst["engine"],
                                "ins": [], "outs": [],
                                "name": f"I-antsw{ctr}",
                                "opcode": "NoOp",
                                "sync_info": {"on_update": [], "on_wait": [w]},
                            })
                            ctr += 1
                        si["on_wait"] = [ow[-1]]
                    out.append(inst)
                blk["instructions"] = out
        return json.dumps(d).encode()

    import concourse.bass_utils as bu
    import concourse.bass2jax as b2j

    orig = bu.compile_bir_kernel

    def patched(bir_json, tmpdir, neff_name="file.neff"):
        return orig(_split_multiwaits(bir_json), tmpdir, neff_name=neff_name)

    bu.compile_bir_kernel = patched
    b2j.compile_bir_kernel = patched


def _build_program(flags):
    nc = bass.Bass("TRN2", target_bir_lowering=False, debug=False,
                   num_devices=NCORES)

    def din(name, shape, dt):
        return nc.dram_tensor(name, shape, dt, kind="ExternalInput").ap()

    xT = din("xT", [D, L], bf16)            # batch x, transposed
    xTq = din("xTq", [D, RQ], bf16)         # this core's columns of x[b].T
    xr = din("xr", [RQ, D], f32)            # this core's rows (residual)
    cosr = din("cosr", [128, L], bf16)      # cos table, [p%32] replicated
    sinr = din("sinr", [128, L], bf16)      # sign-baked sin table
    qcos = din("qcos", [128, RQ], bf16)     # cos table slice for these rows
    qsin = din("qsin", [128, RQ], bf16)     # sign-baked sin slice
    wq = din("wq", [D, D], bf16)
    wk = din("wk", [D, D], bf16)
    wv = din("wv", [D, D], bf16)
    wo = din("wo", [D, D], bf16)
    w1 = din("w1", [D, F], bf16)
    w2 = din("w2", [F, D], bf16)
    b1t = din("b1t", [128, F // 128], f32)  # b1 reshaped per-partition
    ident = din("ident", [128, 128], f32)
    onehot = din("onehot", [2, 128], bf16)
    bo = din("bo", [1, D], f32)
    b2r = din("b2r", [1, D], f32)
    g1 = din("g1", [1, D], f32)
    be1 = din("be1", [1, D], f32)
    g2 = din("g2", [1, D], f32)
    be2 = din("be2", [1, D], f32)
    y = nc.dram_tensor("y", [RQ, D], f32, kind="ExternalOutput").ap()

    def bcast_ap(ap2d, width):
        return bass.AP(tensor=ap2d.tensor, offset=ap2d.offset,
                       ap=[[0, 128], [1, width]])

    with tile.TileContext(nc) as tc:
      with ExitStack() as top:
        consts = top.enter_context(tc.tile_pool(name="consts", bufs=1))
        poolP1 = top.enter_context(tc.tile_pool(name="p1", bufs=1))
        dramp = top.enter_context(tc.tile_pool(name="dramp", bufs=1,
                                               space="DRAM"))
        h_dram = dramp.tile([RQ, D], f32, tag="h_dram", name="h_dram")
        stackCtx = ExitStack()
        poolCtx = stackCtx.enter_context(tc.tile_pool(name="pctx", bufs=1))
        stackP2 = ExitStack()
        poolP2 = stackP2.enter_context(tc.tile_pool(name="p2", bufs=1))
        poolE = stackP2.enter_context(tc.tile_pool(name="pe", bufs=3))
        poolCR = stackP2.enter_context(tc.tile_pool(name="pcr", bufs=1))
        stackP3 = ExitStack()
        poolP3 = stackP3.enter_context(tc.tile_pool(name="p3", bufs=1))
        poolRW = stackP3.enter_context(tc.tile_pool(name="prw", bufs=2))

        _qs = [nc.sync, nc.scalar, nc.gpsimd]
        _qi = [0]

        def ld(dst, src):
            # preamble loads: round-robin all three DMA-capable queues
            _qs[_qi[0] % 3].dma_start(dst, src)
            _qi[0] += 1

        _qs2 = [nc.sync, nc.gpsimd]
        _qi2 = [0]

        def ld2(dst, src):
            # loads issued while the exp stream owns the scalar queue
            _qs2[_qi2[0] % 2].dma_start(dst, src)
            _qi2[0] += 1

        # --- long-lived constants ---
        ident_sb = consts.tile([128, 128], f32, tag="ident", name="ident")
        nc.sync.dma_start(ident_sb[:], ident[:])
        b1_sb = consts.tile([128, F // 128], f32, tag="b1", name="b1")
        nc.scalar.dma_start(b1_sb[:], b1t[:])
        onehot_sb = consts.tile([2, 128], bf16, tag="onehot", name="onehot")
        nc.gpsimd.dma_start(onehot_sb[:], onehot[:])
        eps_sb = consts.tile([128, 1], f32, tag="eps", name="eps")
        nc.vector.memset(eps_sb[:], EPS)

        def rep_const(ap2d, use, tag):
            if not use:
                return None
            t = consts.tile([128, D], f32, tag=tag, name=tag)
            nc.gpsimd.dma_start(out=t[:], in_=bcast_ap(ap2d, D))
            return t

        bo_rep = rep_const(bo, flags["use_bo"], "bo")
        b2_rep = rep_const(b2r, flags["use_b2"], "b2")
        g1_rep = rep_const(g1, flags["use_g1"], "g1")
        be1_rep = rep_const(be1, flags["use_be1"], "be1")
        g2_rep = rep_const(g2, flags["use_g2"], "g2")
        be2_rep = rep_const(be2, flags["use_be2"], "be2")

        # hT persists from the w_o phase into the FFN (top-level pool)
        hT = [poolP1.tile([128, RQ], bf16, tag=f"hT{k}", name=f"hT{k}")
              for k in range(KT)]
        # ctxT persists into the w_o phase only
        ctxT = [poolCtx.tile([128, RQ], bf16, tag=f"ctxT{m}", name=f"ctxT{m}")
                for m in range(KT)]

        # attention-lived arrays
        kTr = [poolP2.tile([128, L], bf16, tag=f"kTr{m}", name=f"kTr{m}")
               for m in range(KT)]
        qTr = [poolP2.tile([128, RQ], bf16, tag=f"qTr{m}", name=f"qTr{m}")
               for m in range(KT)]
        v_lo = [poolP2.tile([128, 8 * VSTR], bf16, tag=f"vl{t}",
                            name=f"vl{t}") for t in range(LT)]
        v_hi = [poolP2.tile([128, 8 * VSTR], bf16, tag=f"vh{t}",
                            name=f"vh{t}") for t in range(LT)]
        ctxraw = [poolP2.tile([VSTR, RQ], bf16, tag=f"cr{h}", name=f"cr{h}")
                  for h in range(H)]

        # projection inputs
        xT_sb = [poolP3.tile([128, L], bf16, tag=f"xT{k}", name=f"xT{k}")
                 for k in range(KT)]
        xTq_sb = [poolP3.tile([128, RQ], bf16, tag=f"xTq{k}", name=f"xTq{k}")
                  for k in range(KT)]
        wk_sb = [poolP3.tile([128, D], bf16, tag=f"wk{k}", name=f"wk{k}")
                 for k in range(KT)]
        cos_sb = poolP3.tile([128, L], bf16, tag="cos", name="cos")
        sin_sb = poolP3.tile([128, L], bf16, tag="sin", name="sin")
        qcos_sb = poolP3.tile([128, RQ], bf16, tag="qcos", name="qcos")
        qsin_sb = poolP3.tile([128, RQ], bf16, tag="qsin", name="qsin")
        stackP3b = ExitStack()
        poolWQ = stackP3b.enter_context(tc.tile_pool(name="pwq", bufs=1))
        wq_sb = [poolWQ.tile([128, D], bf16, tag=f"wq{k}", name=f"wq{k}")
                 for k in range(KT)]

        # load order: k-proj inputs first (they gate the exp stream)
        for k in range(KT):
            ld(xT_sb[k][:], xT[k * 128:(k + 1) * 128, :])
            ld(wk_sb[k][:], wk[k * 128:(k + 1) * 128, :])
        ld(cos_sb[:], cosr[:])
        ld(sin_sb[:], sinr[:])
        for k in range(KT):
            ld(wq_sb[k][:], wq[k * 128:(k + 1) * 128, :])
            ld(xTq_sb[k][:], xTq[k * 128:(k + 1) * 128, :])
        ld(qcos_sb[:], qcos[:])
        ld(qsin_sb[:], qsin[:])

        def rope_chunk(ps, cos_sl, sin_sl, dst):
            """dst = ps*cos + swap32(ps*sin) (sign-baked sin table)."""
            tct = poolRW.tile([128, 512], bf16, tag="rtc", name="rtc")
            nc.vector.tensor_mul(tct[:], ps, cos_sl)
            tsn = poolRW.tile([128, 512], bf16, tag="rtm", name="rtm")
            nc.vector.tensor_mul(tsn[:], ps, sin_sl)
            tsw = poolRW.tile([128, 512], bf16, tag="tsw", name="tsw")
            nc.vector.stream_shuffle(tsw[:], tsn[:],
                                     [(i + 16) % 32 for i in range(32)])
            nc.vector.tensor_add(dst, tct[:], tsw[:])

        with tc.tile_pool(name="pjps", bufs=2, space="PSUM") as pjps, \
             tc.tile_pool(name="scps", bufs=2, space="PSUM") as scps, \
             tc.tile_pool(name="ctxps", bufs=1, space="PSUM") as ctxps:

            def q_proj(m):
                msl = slice(m * 128, m * 128 + 128)
                ps = pjps.tile([128, 512], f32, tag="pj", name="pj")
                for k in range(KT):
                    nc.tensor.matmul(ps[:], wq_sb[k][:, msl], xTq_sb[k][:],
                                     start=(k == 0), stop=(k == KT - 1))
                rope_chunk(ps[:], qcos_sb[:], qsin_sb[:], qTr[m][:])

            def k_proj(m, n):
                msl = slice(m * 128, m * 128 + 128)
                nsl = slice(n * 512, n * 512 + 512)
                ps = pjps.tile([128, 512], f32, tag="pj", name="pj")
                for k in range(KT):
                    nc.tensor.matmul(ps[:], wk_sb[k][:, msl],
                                     xT_sb[k][:, nsl],
                                     start=(k == 0), stop=(k == KT - 1))
                rope_chunk(ps[:], cos_sb[:, nsl], sin_sb[:, nsl],
                           kTr[m][:, nsl])

            # kTr[0] first (gates the exp stream), then q projection
            for n in range(NL):
                k_proj(0, n)
            for m in range(KT):
                q_proj(m)
            stackP3b.close()
            poolWV = stackP3.enter_context(
                tc.tile_pool(name="pwv", bufs=1))
            wv_sb = [poolWV.tile([128, D], bf16, tag=f"wv{k}",
                                 name=f"wv{k}") for k in range(KT)]
            for k in range(KT):
                ld(wv_sb[k][:], wv[k * 128:(k + 1) * 128, :])

            def v_proj(half, t):
                tsl = slice(t * 128, t * 128 + 128)
                vt = v_lo[t] if half == 0 else v_hi[t]
                ps = pjps.tile([128, 512], f32, tag="pj", name="pj")
                for k in range(KT):
                    nc.tensor.matmul(ps[:], xT_sb[k][:, tsl],
                                     wv_sb[k][:, half * 512:half * 512 + 512],
                                     start=(k == 0), stop=(k == KT - 1))
                vt2 = v_lo[t] if half == 0 else v_hi[t]
                vview = vt2[:].rearrange("p (h e) -> p h e", h=8)
                ps_view = ps[:].rearrange("p (h e) -> p h e", h=8)
                nc.vector.tensor_copy(vview[:, :, 0:DK], ps_view[:])
                nc.vector.memset(vview[:, :, DK:DK + 1], 1.0)

            # deferred projection work, emitted interleaved into attention
            stream = []
            for t in range(LT):
                stream.append(("v0", 0, t))
            for m in range(1, 5):
                for n in range(NL):
                    stream.append(("k", m, n))
            for t in range(LT):
                stream.append(("v1", 1, t))
            for m in range(5, KT):
                for n in range(NL):
                    stream.append(("k", m, n))
            v_pos = {}   # (half, t) -> stream index
            k_pos = {}   # m -> index of last chunk of k m
            for i, it in enumerate(stream):
                if it[0] == "k":
                    k_pos[it[1]] = i
                else:
                    v_pos[(it[1], it[2])] = i
            fed = [0]

            def emit(it):
                if it[0] == "k":
                    k_proj(it[1], it[2])
                else:
                    v_proj(it[1], it[2])

            def feed(n):
                for _ in range(n):
                    if fed[0] < len(stream):
                        emit(stream[fed[0]])
                        fed[0] += 1

            def drain_to(idx):
                while fed[0] <= idx:
                    emit(stream[fed[0]])
                    fed[0] += 1

            # ---- attention (projection stream fed between steps) ----
            for hp in range(KT):
                if hp >= 1:
                    drain_to(k_pos[hp])
                hA, hB = 2 * hp, 2 * hp + 1
                half = 0 if hp < 4 else 1
                vt = v_lo if hp < 4 else v_hi
                ca = (hA % 8) * VSTR
                cb = (hB % 8) * VSTR
                cpsA = ctxps.tile([VSTR, RQ], f32, tag="cpsA", name="cpsA")
                cpsB = ctxps.tile([VSTR, RQ], f32, tag="cpsB", name="cpsB")
                pend = None
                for kt in range(LT):
                    off = kt * 128
                    sc = scps.tile([128, 2 * RQ], f32, tag="sc", name="sc")
                    nc.tensor.matmul(sc[:, 0:RQ],
                                     kTr[hp][0:64, off:off + 128],
                                     qTr[hp][0:64, :], start=True, stop=True)
                    nc.tensor.matmul(sc[:, RQ:2 * RQ],
                                     kTr[hp][64:128, off:off + 128],
                                     qTr[hp][64:128, :], start=True, stop=True)
                    e = poolE.tile([128, 2 * RQ], bf16, tag="e", name="e")
                    nc.scalar.activation(e[:], sc[:], AF.Exp, scale=0.125)
                    feed(1)
                    if pend is not None:
                        ep, ktp = pend
                        nc.tensor.matmul(cpsA[:], vt[ktp][:, ca:ca + VSTR],
                                         ep[:, 0:RQ],
                                         start=(ktp == 0), stop=False)
                        nc.tensor.matmul(cpsB[:], vt[ktp][:, cb:cb + VSTR],
                                         ep[:, RQ:2 * RQ],
                                         start=(ktp == 0), stop=False)
                    drain_to(v_pos[(half, min(kt + 1, LT - 1))])
                    pend = (e, kt)
                ep, ktp = pend
                nc.tensor.matmul(cpsA[:], vt[ktp][:, ca:ca + VSTR],
                                 ep[:, 0:RQ], start=False, stop=True)
                nc.tensor.matmul(cpsB[:], vt[ktp][:, cb:cb + VSTR],
                                 ep[:, RQ:2 * RQ], start=False, stop=True)
                nc.vector.tensor_copy(ctxraw[hA][:], cpsA[:])
                nc.vector.tensor_copy(ctxraw[hB][:], cpsB[:])
                s2 = poolCR.tile([2, RQ], bf16, tag="s2", name="s2")
                nc.sync.dma_start(s2[0:1, :], ctxraw[hA][64:65, :])
                nc.sync.dma_start(s2[1:2, :], ctxraw[hB][64:65, :])
                rec2 = poolCR.tile([2, RQ], f32, tag="rc", name="rc")
                nc.vector.reciprocal(rec2[:], s2[:])
                rcb = poolCR.tile([2, RQ], bf16, tag="rcb", name="rcb")
                nc.vector.tensor_copy(rcb[:], rec2[:])
                for h, cr in ((hA, ctxraw[hA]), (hB, ctxraw[hB])):
                    half2 = h % 2
                    rp = pjps.tile([128, 512], f32, tag="pj", name="pj")
                    nc.tensor.matmul(
                        rp[0:64, :],
                        onehot_sb[0:2, half2 * 64:half2 * 64 + 64],
                        rcb[:], start=True, stop=True)
                    dst = ctxT[hp][half2 * 64:half2 * 64 + 64, :]
                    nc.vector.tensor_mul(dst, cr[0:64, :], rp[0:64, :])

            stackP3.close()

        stackP2.close()

        # layer norm helper (takes its workspace pool)
        def layer_norm(dst, src, g_rep, be_rep, wpool):
            sview = src.rearrange("p (s d) -> p s d", s=2)
            stats = wpool.tile([128, 2, 6], f32, tag="lnstats",
                               name="lnstats")
            for sg in range(2):
                nc.vector.bn_stats(stats[:, sg, :], sview[:, sg, :])
            mv = wpool.tile([128, 2], f32, tag="lnmv", name="lnmv")
            nc.vector.bn_aggr(mv[:], stats[:])
            std = wpool.tile([128, 1], f32, tag="lnstd", name="lnstd")
            nc.scalar.activation(std[:], mv[:, 1:2], AF.Sqrt, bias=eps_sb[:])
            rstd = wpool.tile([128, 1], f32, tag="lnrstd", name="lnrstd")
            nc.vector.reciprocal(rstd[:], std[:])
            nc.vector.tensor_scalar(dst, src, mv[:, 0:1], rstd[:],
                                    op0=ALU.subtract, op1=ALU.mult)
            if g_rep is not None:
                nc.vector.tensor_mul(dst, dst, g_rep[:])
            if be_rep is not None:
                nc.vector.tensor_add(dst, dst, be_rep[:])

        # ---- w_o + residual + LN1 + transpose (per query tile) ----
        with tc.tile_pool(name="pwo", bufs=1) as poolWO, \
             tc.tile_pool(name="ph3w", bufs=2) as ph3w, \
             tc.tile_pool(name="aops", bufs=2, space="PSUM") as aops, \
             tc.tile_pool(name="tpps", bufs=4, space="PSUM") as tpps:
            wo_sb = [poolWO.tile([128, D], bf16, tag=f"wo{k}", name=f"wo{k}")
                     for k in range(KT)]
            xr_sb = [poolWO.tile([128, D], f32, tag=f"xr{t}", name=f"xr{t}")
                     for t in range(4)]
            for k in range(KT):
                ld2(wo_sb[k][:], wo[k * 128:(k + 1) * 128, :])
            for t in range(4):
                ld2(xr_sb[t][:], xr[t * 128:(t + 1) * 128, :])
            for qt in range(4):
                qsl = slice(qt * 128, qt * 128 + 128)
                ps = aops.tile([128, D], f32, tag="ao", name="ao")
                for half in range(2):
                    osl = slice(half * 512, half * 512 + 512)
                    for m in range(KT):
                        nc.tensor.matmul(ps[:, osl], ctxT[m][:, qsl],
                                         wo_sb[m][:, osl],
                                         start=(m == 0), stop=(m == KT - 1))
                res = ph3w.tile([128, D], f32, tag="res", name="res")
                nc.vector.tensor_add(res[:], ps[:], xr_sb[qt][:])
                if bo_rep is not None:
                    nc.vector.tensor_add(res[:], res[:], bo_rep[:])
                hq = ph3w.tile([128, D], f32, tag="hq", name="hq")
                layer_norm(hq[:], res[:], g1_rep, be1_rep, ph3w)
                nc.sync.dma_start(h_dram[qsl, :], hq[:])
                for m in range(KT):
                    tp = tpps.tile([128, 128], f32, tag="tp", name="tp")
                    nc.tensor.transpose(tp[:], hq[:, m * 128:m * 128 + 128],
                                        ident_sb[:])
                    nc.vector.tensor_copy(hT[m][:, qsl], tp[:])

        stackCtx.close()

        # ---- FFN (weights into all the freed space) ----
        with tc.tile_pool(name="pffn", bufs=1) as pffn, \
             tc.tile_pool(name="ph4w", bufs=2) as ph4w:
            ff1rT = [pffn.tile([128, RQ], bf16, tag=f"ff1{t}",
                               name=f"ff1{t}") for t in range(FT)]
            w1_sb = [pffn.tile([128, F], bf16, tag=f"w1{k}", name=f"w1{k}")
                     for k in range(KT)]
            w2_sb = [pffn.tile([128, D], bf16, tag=f"w2{k}", name=f"w2{k}")
                     for k in range(FT)]
            # w1 column-block-major so ffn1 chain ft=0 starts early
            for g in range(8):
                gsl = slice(g * 512, (g + 1) * 512)
                for k in range(KT):
                    ld2(w1_sb[k][:, gsl], w1[k * 128:(k + 1) * 128, gsl])
            # w2 half-column-major
            for half in range(2):
                osl = slice(half * 512, half * 512 + 512)
                for k in range(FT):
                    ld2(w2_sb[k][:, osl], w2[k * 128:(k + 1) * 128, osl])

            # ---- FFN up + ReLU ----
            with tc.tile_pool(name="f1ps", bufs=4, space="PSUM") as f1ps:
                for ft in range(FT):
                    fsl = slice(ft * 128, ft * 128 + 128)
                    ps = f1ps.tile([128, RQ], f32, tag="f1", name="f1")
                    for k in range(KT):
                        nc.tensor.matmul(ps[:], w1_sb[k][:, fsl], hT[k][:],
                                         start=(k == 0), stop=(k == KT - 1))
                    nc.scalar.activation(ff1rT[ft][:], ps[:], AF.Relu,
                                         bias=b1_sb[:, ft:ft + 1])

            # ---- FFN down + LN2 ----
            with tc.tile_pool(name="f2ps", bufs=3, space="PSUM") as f2ps:
                for qt in range(4):
                    qsl = slice(qt * 128, qt * 128 + 128)
                    ps = f2ps.tile([128, D], f32, tag="f2", name="f2")
                    for half in range(2):
                        osl = slice(half * 512, half * 512 + 512)
                        for ft in range(FT):
                            nc.tensor.matmul(ps[:, osl], ff1rT[ft][:, qsl],
                                             w2_sb[ft][:, osl],
                                             start=(ft == 0),
                                             stop=(ft == FT - 1))
                    hback = ph4w.tile([128, D], f32, tag="hback",
                                      name="hback")
                    nc.sync.dma_start(hback[:], h_dram[qsl, :])
                    res = ph4w.tile([128, D], f32, tag="res2", name="res2")
                    nc.vector.tensor_add(res[:], ps[:], hback[:])
                    if b2_rep is not None:
                        nc.vector.tensor_add(res[:], res[:], b2_rep[:])
                    o = ph4w.tile([128, D], f32, tag="out", name="out")
                    layer_norm(o[:], res[:], g2_rep, be2_rep, ph4w)
                    nc.sync.dma_start(y[qt * 128:(qt + 1) * 128, :], o[:])

    return nc


_CACHED = {}


def _get_program(flags):
    key = tuple(sorted(flags.items()))
    if key not in _CACHED:
        _CACHED[key] = _build_program(flags)
    return _CACHED[key]


def kernel(x, w_q, w_k, w_v, w_o, b_o, gamma1, beta1, gamma2, beta2,
           w1, b1, w2, b2, _trace=False):
    _install_patches()
    from concourse import bass_utils

    bf = ml_dtypes.bfloat16
    x = np.asarray(x, np.float32)
    flags = {
        "use_bo": not np.all(np.asarray(b_o) == 0),
        "use_b2": not np.all(np.asarray(b2) == 0),
        "use_g1": not np.all(np.asarray(gamma1) == 1),
        "use_be1": not np.all(np.asarray(beta1) == 0),
        "use_g2": not np.all(np.asarray(gamma2) == 1),
        "use_be2": not np.all(np.asarray(beta2) == 0),
    }
    nc = _get_program(flags)

    # host-side shared prep. Head dims are permuted so the RoPE partner
    # (d <-> d+32) sits 16 partitions away within the same 32-partition
    # quadrant, making the partner swap a DVE stream_shuffle:
    #   new position p (within a 64-dim head) holds old dim PI[p].
    PI = np.concatenate([np.arange(0, 16), np.arange(32, 48),
                         np.arange(16, 32), np.arange(48, 64)])
    inv_freq = (1.0 / (10000.0 ** (np.arange(0, DK, 2, dtype=np.float64) / DK)))
    freqs = np.arange(L, dtype=np.float64)[:, None] * inv_freq      # [L, 32]
    cos = np.cos(freqs).T.astype(np.float32)                        # [32, L]
    sin = np.sin(freqs).T.astype(np.float32)
    fidx = PI % 32                     # frequency index per new position
    sgn = np.where(PI < 32, 1.0, -1.0).astype(np.float32)[:, None]
    cos_rep = np.concatenate([cos[fidx % 32], cos[fidx % 32]],
                             0).astype(bf)                          # [128, L]
    sin_sign = np.concatenate([sgn * sin[fidx], sgn * sin[fidx]],
                              0).astype(bf)

    def permute_heads(w):
        # permute output-dim columns of a [D, D] projection weight
        wp = np.asarray(w, np.float32).reshape(D, H, DK)
        return np.ascontiguousarray(wp[:, :, PI].reshape(D, D))

    common = {
        "cosr": cos_rep, "sinr": sin_sign,
        "wq": permute_heads(w_q).astype(bf),
        "wk": permute_heads(w_k).astype(bf), "wv": w_v.astype(bf),
        "wo": w_o.astype(bf), "w1": w1.astype(bf), "w2": w2.astype(bf),
        "b1t": np.ascontiguousarray(
            np.asarray(b1, np.float32).reshape(F // 128, 128).T),
        "ident": np.eye(128, dtype=np.float32),
        "onehot": np.concatenate(
            [np.concatenate([np.ones((1, 64), np.float32),
                             np.zeros((1, 64), np.float32)], 1),
             np.concatenate([np.zeros((1, 64), np.float32),
                             np.ones((1, 64), np.float32)], 1)],
            0).astype(bf),
        "bo": np.asarray(b_o, np.float32).reshape(1, D),
        "b2r": np.asarray(b2, np.float32).reshape(1, D),
        "g1": np.asarray(gamma1, np.float32).reshape(1, D),
        "be1": np.asarray(beta1, np.float32).reshape(1, D),
        "g2": np.asarray(gamma2, np.float32).reshape(1, D),
        "be2": np.asarray(beta2, np.float32).reshape(1, D),
    }
    xT_all = [np.ascontiguousarray(x[b].T).astype(bf) for b in range(B)]

    in_maps = []
    for c in range(NCORES):
        b, r = c // 4, c % 4
        rows = slice(r * RQ, (r + 1) * RQ)
        m = dict(common)
        m["xT"] = xT_all[b]
        m["xTq"] = np.ascontiguousarray(xT_all[b][:, rows])
        m["xr"] = np.ascontiguousarray(x[b, rows, :])
        m["qcos"] = np.ascontiguousarray(cos_rep[:, rows])
        m["qsin"] = np.ascontiguousarray(sin_sign[:, rows])
        in_maps.append(m)

    res = bass_utils.run_bass_kernel_spmd(
        nc, in_maps, core_ids=list(range(NCORES)), trace=_trace)

    out = np.empty((B, L, D), np.float32)
    for c in range(NCORES):
        b, r = c // 4, c % 4
        out[b, r * RQ:(r + 1) * RQ, :] = res.results[c]["y"]
    if _trace:
        kernel.last_exec_time_ns = res.exec_time_ns
    return out


# revision 15
# speedup vs baseline: 1.2125x; 1.1603x over previous
"""Trainium2 Bass kernel for a dense transformer encoder layer.

Shapes (hardcoded): B=2, L=2048, D=1024, F=4096, H=16 heads, dk=64.
Sharding over 8 NeuronCores: core c handles batch b=c//4 and query-row
quarter r=c%4 (512 rows). K/V projections for the full batch are
computed per core (replicated within the 4-core batch group; collectives
measured too slow here due to cross-core launch skew).

Performance structure:
- Projections are fused INTO the attention loop: the softmax exp stream
  (scalar engine) is attention's long pole, and K/V projection matmuls
  are emitted interleaved with score/context matmuls.
- Q/K/V/O projections and the attention context matmul run in fp8
  (e4m3) with DoubleRow perf mode (2 contraction tiles per matmul,
  ~1.4x tensor throughput). Weights are pre-scaled by 64 (w_v by 32)
  to stay in fp8 normal range; the scale is compensated for free in the
  RoPE tables (/64), the softmax ones-column (=32), and the w_o
  residual add (x 1/64). The FFN stays bf16 (fp8 there costs too much
  accuracy).
- RoPE's partner swap is a DVE stream_shuffle: head dims are permuted
  host-side so partners sit 16 partitions apart within one quadrant.
- Rescale is fused per head-pair; the epilogue (w_o + LN1 + FFN + LN2)
  is pipelined per query tile.
"""
import os
import sys
import types

sys.path.insert(0, "/opt/trn_rl_repo")

import numpy as np
import ml_dtypes

import concourse.bass as bass
import concourse.tile as tile
import concourse.mybir as mybir
from contextlib import ExitStack

f32 = mybir.dt.float32
bf16 = mybir.dt.bfloat16
f8 = mybir.dt.float8e4
AF = mybir.ActivationFunctionType
ALU = mybir.AluOpType
DR = mybir.MatmulPerfMode.DoubleRow

B, L, D, F, H, DK = 2, 2048, 1024, 4096, 16, 64
RQ = 512          # query rows per core
NCORES = 8
EPS = 1e-6
KT = D // 128      # 8 contraction tiles over D
KP = KT // 2       # 4 DoubleRow contraction pairs
NL = L // 512      # 4 column chunks over L
LT = L // 128      # 16 key tiles
FT = F // 128      # 32 f-tiles
VSTR = 65          # per-head stride in v tiles (64 v cols + ones)
VPAD = 528         # fp8 pair stride for v tiles (16-aligned)

_PATCHED = False


def _install_patches():
    """Register the NTFF profile hook (if available) and wrap the BIR
    compile step to split multi-wait instructions (this walrus build
    accepts at most one sync-wait per instruction)."""
    global _PATCHED
    if _PATCHED:
        return
    _PATCHED = True

    if "antenv.axon_hooks" not in sys.modules:
        try:
            from trn_agent_boot.trn_boot import _ntff_profile_via_ctypes
            hook = _ntff_profile_via_ctypes("/opt/axon/libaxon_pjrt.so")
        except Exception:
            hook = None
        mod = types.ModuleType("antenv.axon_hooks")
        mod.get_axon_ntff_profile_hook = lambda: hook
        mod.set_axon_ntff_profile_hook = lambda h: None
        sys.modules["antenv.axon_hooks"] = mod

    import json

    def _split_multiwaits(bir_bytes):
        d = json.loads(bir_bytes)
        ctr = 0
        for fn in d.get("functions", []):
            for blk in fn.get("blocks", []):
                out = []
                for inst in blk.get("instructions", []):
                    si = inst.get("sync_info")
                    ow = (si or {}).get("on_wait") or []
                    if len(ow) > 1 and inst.get("engine", "Unassigned") != "Unassigned":
                        for w in ow[:-1]:
                            out.append({
                                "debug": inst.get("debug", 0),
                                "engine": inst["engine"],
                                "ins": [], "outs": [],
                                "name": f"I-antsw{ctr}",
                                "opcode": "NoOp",
                                "sync_info": {"on_update": [], "on_wait": [w]},
                            })
                            ctr += 1
                        si["on_wait"] = [ow[-1]]
                    out.append(inst)
                blk["instructions"] = out
        return json.dumps(d).encode()

    import concourse.bass_utils as bu
    import concourse.bass2jax as b2j

    orig = bu.compile_bir_kernel

    def patched(bir_json, tmpdir, neff_name="file.neff"):
        return orig(_split_multiwaits(bir_json), tmpdir, neff_name=neff_name)

    bu.compile_bir_kernel = patched
    b2j.compile_bir_kernel = patched


def _build_program(flags):
    nc = bass.Bass("TRN2", target_bir_lowering=False, debug=False,
                   num_devices=NCORES)

    def din(name, shape, dt):
        return nc.dram_tensor(name, shape, dt, kind="ExternalInput").ap()

    xT = din("xT", [D, L], f8)              # batch x, transposed, fp8
    xTq = din("xTq", [D, RQ], f8)           # this core's columns of x[b].T
    xr = din("xr", [RQ, D], f32)            # this core's rows (residual)
    cosr = din("cosr", [128, L], bf16)      # cos table (permuted, /64)
    sinr = din("sinr", [128, L], bf16)      # sign-baked sin table (/64)
    qcos = din("qcos", [128, RQ], bf16)
    qsin = din("qsin", [128, RQ], bf16)
    wq = din("wq", [D, D], f8)              # x64, head-dims permuted
    wk = din("wk", [D, D], f8)              # x64, head-dims permuted
    wv = din("wv", [D, D], f8)              # x32
    wo = din("wo", [D, D], f8)              # x64
    w1 = din("w1", [D, F], bf16)
    w2 = din("w2", [F, D], bf16)
    b1t = din("b1t", [128, F // 128], f32)
    ident = din("ident", [128, 128], f32)
    onehot = din("onehot", [2, 128], bf16)
    bo = din("bo", [1, D], f32)
    b2r = din("b2r", [1, D], f32)
    g1 = din("g1", [1, D], f32)
    be1 = din("be1", [1, D], f32)
    g2 = din("g2", [1, D], f32)
    be2 = din("be2", [1, D], f32)
    y = nc.dram_tensor("y", [RQ, D], f32, kind="ExternalOutput").ap()

    def bcast_ap(ap2d, width):
        return bass.AP(tensor=ap2d.tensor, offset=ap2d.offset,
                       ap=[[0, 128], [1, width]])

    def pairv(t):
        # [128, 2*W] tile -> [128, 2, W] DoubleRow view
        return t[:].rearrange("p (a b) -> p a b", a=2)

    with tile.TileContext(nc) as tc:
      with ExitStack() as top:
        consts = top.enter_context(tc.tile_pool(name="consts", bufs=1))
        poolP1 = top.enter_context(tc.tile_pool(name="p1", bufs=1))
        dramp = top.enter_context(tc.tile_pool(name="dramp", bufs=1,
                                               space="DRAM"))
        h_dram = dramp.tile([RQ, D], f32, tag="h_dram", name="h_dram")
        stackCtx = ExitStack()
        poolCtx = stackCtx.enter_context(tc.tile_pool(name="pctx", bufs=1))
        stackP2 = ExitStack()
        poolP2 = stackP2.enter_context(tc.tile_pool(name="p2", bufs=1))
        poolE = stackP2.enter_context(tc.tile_pool(name="pe", bufs=3))
        poolCR = stackP2.enter_context(tc.tile_pool(name="pcr", bufs=2))
        stackP3 = ExitStack()
        poolP3 = stackP3.enter_context(tc.tile_pool(name="p3", bufs=1))
        poolRW = stackP3.enter_context(tc.tile_pool(name="prw", bufs=2))

        _qs = [nc.sync, nc.scalar, nc.gpsimd]
        _qi = [0]

        def ld(dst, src):
            _qs[_qi[0] % 3].dma_start(dst, src)
            _qi[0] += 1

        _qs2 = [nc.sync, nc.gpsimd]
        _qi2 = [0]

        def ld2(dst, src):
            # loads issued while the exp stream owns the scalar queue
            _qs2[_qi2[0] % 2].dma_start(dst, src)
            _qi2[0] += 1

        # --- long-lived constants ---
        ident_sb = consts.tile([128, 128], f32, tag="ident", name="ident")
        nc.sync.dma_start(ident_sb[:], ident[:])
        b1_sb = consts.tile([128, F // 128], f32, tag="b1", name="b1")
        nc.scalar.dma_start(b1_sb[:], b1t[:])
        onehot_sb = consts.tile([2, 128], bf16, tag="onehot", name="onehot")
        nc.gpsimd.dma_start(onehot_sb[:], onehot[:])
        eps_sb = consts.tile([128, 1], f32, tag="eps", name="eps")
        nc.vector.memset(eps_sb[:], EPS)

        def rep_const(ap2d, use, tag):
            if not use:
                return None
            t = consts.tile([128, D], f32, tag=tag, name=tag)
            nc.gpsimd.dma_start(out=t[:], in_=bcast_ap(ap2d, D))
            return t

        bo_rep = rep_const(bo, flags["use_bo"], "bo")
        b2_rep = rep_const(b2r, flags["use_b2"], "b2")
        g1_rep = rep_const(g1, flags["use_g1"], "g1")
        be1_rep = rep_const(be1, flags["use_be1"], "be1")
        g2_rep = rep_const(g2, flags["use_g2"], "g2")
        be2_rep = rep_const(be2, flags["use_be2"], "be2")

        # hT persists from the w_o phase into the FFN (top-level pool)
        hT = [poolP1.tile([128, RQ], bf16, tag=f"hT{k}", name=f"hT{k}")
              for k in range(KT)]
        # ctxT pairs (fp8, DoubleRow stationary for w_o)
        ctxTp = [poolCtx.tile([128, 2 * RQ], f8, tag=f"ctxT{j}",
                              name=f"ctxT{j}") for j in range(KP)]

        # attention-lived arrays
        kTr = [poolP2.tile([128, L], bf16, tag=f"kTr{m}", name=f"kTr{m}")
               for m in range(KT)]
        qTr = [poolP2.tile([128, RQ], bf16, tag=f"qTr{m}", name=f"qTr{m}")
               for m in range(KT)]
        # v pairs (fp8, DoubleRow stationary for ctx); halves at 0/VPAD
        vpl = [poolP2.tile([128, 2 * VPAD], f8, tag=f"vl{i}", name=f"vl{i}")
               for i in range(KT)]
        vph = [poolP2.tile([128, 2 * VPAD], f8, tag=f"vh{i}", name=f"vh{i}")
               for i in range(KT)]
        ctxraw = [poolP2.tile([VSTR, RQ], bf16, tag=f"cr{h}", name=f"cr{h}")
                  for h in range(H)]

        # projection inputs (fp8 pairs for DoubleRow)
        xtp = [poolP3.tile([128, 2 * L], f8, tag=f"xt{j}", name=f"xt{j}")
               for j in range(KP)]
        xqp = [poolP3.tile([128, 2 * RQ], f8, tag=f"xq{j}", name=f"xq{j}")
               for j in range(KP)]
        wkp = [poolP3.tile([128, 2 * D], f8, tag=f"wk{j}", name=f"wk{j}")
               for j in range(KP)]
        wqp = [poolP3.tile([128, 2 * D], f8, tag=f"wq{j}", name=f"wq{j}")
               for j in range(KP)]
        wvp = [poolP3.tile([128, 2 * D], f8, tag=f"wv{j}", name=f"wv{j}")
               for j in range(KP)]
        cos_sb = poolP3.tile([128, L], bf16, tag="cos", name="cos")
        sin_sb = poolP3.tile([128, L], bf16, tag="sin", name="sin")
        qcos_sb = poolP3.tile([128, RQ], bf16, tag="qcos", name="qcos")
        qsin_sb = poolP3.tile([128, RQ], bf16, tag="qsin", name="qsin")

        def ld_pair(dsts, src, width):
            for j in range(KP):
                ld(dsts[j][:, 0:width], src[(2 * j) * 128:(2 * j + 1) * 128, :])
                ld(dsts[j][:, width:2 * width],
                   src[(2 * j + 1) * 128:(2 * j + 2) * 128, :])

        # load order: k-proj inputs first (they gate the exp stream)
        ld_pair(xtp, xT, L)
        ld_pair(wkp, wk, D)
        ld(cos_sb[:], cosr[:])
        ld(sin_sb[:], sinr[:])
        ld_pair(wqp, wq, D)
        ld_pair(xqp, xTq, RQ)
        ld(qcos_sb[:], qcos[:])
        ld(qsin_sb[:], qsin[:])
        ld_pair(wvp, wv, D)

        xtv = [pairv(t) for t in xtp]
        xqv = [pairv(t) for t in xqp]
        wkv = [pairv(t) for t in wkp]
        wqv = [pairv(t) for t in wqp]
        wvv = [pairv(t) for t in wvp]

        def rope_chunk(ps, cos_sl, sin_sl, dst):
            """dst = ps*cos + shuffle16(ps*sin); tables carry the 1/64."""
            tct = poolRW.tile([128, 512], bf16, tag="rtc", name="rtc")
            nc.vector.tensor_mul(tct[:], ps, cos_sl)
            tsn = poolRW.tile([128, 512], bf16, tag="rtm", name="rtm")
            nc.vector.tensor_mul(tsn[:], ps, sin_sl)
            tsw = poolRW.tile([128, 512], bf16, tag="tsw", name="tsw")
            nc.vector.stream_shuffle(tsw[:], tsn[:],
                                     [(i + 16) % 32 for i in range(32)])
            nc.vector.tensor_add(dst, tct[:], tsw[:])

        with tc.tile_pool(name="pjps", bufs=2, space="PSUM") as pjps, \
             tc.tile_pool(name="scps", bufs=2, space="PSUM") as scps, \
             tc.tile_pool(name="ctxps", bufs=1, space="PSUM") as ctxps:

            def q_proj(m):
                msl = slice(m * 128, m * 128 + 128)
                ps = pjps.tile([128, 512], f32, tag="pj", name="pj")
                for j in range(KP):
                    nc.tensor.matmul(ps[:], wqv[j][:, :, msl], xqv[j][:],
                                     start=(j == 0), stop=(j == KP - 1),
                                     perf_mode=DR)
                rope_chunk(ps[:], qcos_sb[:], qsin_sb[:], qTr[m][:])

            def k_proj(m, n):
                msl = slice(m * 128, m * 128 + 128)
                nsl = slice(n * 512, n * 512 + 512)
                ps = pjps.tile([128, 512], f32, tag="pj", name="pj")
                for j in range(KP):
                    nc.tensor.matmul(ps[:], wkv[j][:, :, msl],
                                     xtv[j][:, :, nsl],
                                     start=(j == 0), stop=(j == KP - 1),
                                     perf_mode=DR)
                rope_chunk(ps[:], cos_sb[:, nsl], sin_sb[:, nsl],
                           kTr[m][:, nsl])

            # kTr[0] first (gates the exp stream), then q projection
            for n in range(NL):
                k_proj(0, n)
            for m in range(KT):
                q_proj(m)

            def v_proj(half, t):
                tsl = slice(t * 128, t * 128 + 128)
                ps = pjps.tile([128, 512], f32, tag="pj", name="pj")
                for j in range(KP):
                    nc.tensor.matmul(ps[:], xtv[j][:, :, tsl],
                                     wvv[j][:, :, half * 512:half * 512 + 512],
                                     start=(j == 0), stop=(j == KP - 1),
                                     perf_mode=DR)
                vp = (vpl if half == 0 else vph)[t // 2]
                off = (t % 2) * VPAD
                vview = vp[:, off:off + 8 * VSTR].rearrange(
                    "p (h e) -> p h e", h=8)
                ps_view = ps[:].rearrange("p (h e) -> p h e", h=8)
                nc.vector.tensor_copy(vview[:, :, 0:DK], ps_view[:])
                # ones column carries the 1/32 compensation for wv's x32
                nc.vector.memset(vview[:, :, DK:DK + 1], 32.0)

            # deferred projection work, emitted interleaved into attention
            stream = []
            for t in range(LT):
                stream.append(("v0", 0, t))
            for m in range(1, 5):
                for n in range(NL):
                    stream.append(("k", m, n))
            for t in range(LT):
                stream.append(("v1", 1, t))
            for m in range(5, KT):
                for n in range(NL):
                    stream.append(("k", m, n))
            v_pos = {}
            k_pos = {}
            for i, it in enumerate(stream):
                if it[0] == "k":
                    k_pos[it[1]] = i
                else:
                    v_pos[(it[1], it[2])] = i
            fed = [0]

            def emit(it):
                if it[0] == "k":
                    k_proj(it[1], it[2])
                else:
                    v_proj(it[1], it[2])

            def feed(n):
                for _ in range(n):
                    if fed[0] < len(stream):
                        emit(stream[fed[0]])
                        fed[0] += 1

            def drain_to(idx):
                while fed[0] <= idx:
                    emit(stream[fed[0]])
                    fed[0] += 1

            # ---- attention (projection stream fed between steps) ----
            for hp in range(KT):
                if hp >= 1:
                    drain_to(k_pos[hp])
                hA, hB = 2 * hp, 2 * hp + 1
                half = 0 if hp < 4 else 1
                vt = vpl if hp < 4 else vph
                ca = (hA % 8) * VSTR
                cb = (hB % 8) * VSTR
                cpsA = ctxps.tile([VSTR, RQ], f32, tag="cpsA", name="cpsA")
                cpsB = ctxps.tile([VSTR, RQ], f32, tag="cpsB", name="cpsB")

                def emit_ctx(ep, i):
                    epv = pairv(ep)
                    vv = pairv(vt[i])
                    nc.tensor.matmul(cpsA[:], vv[:, :, ca:ca + VSTR],
                                     epv[:, :, 0:RQ],
                                     start=(i == 0), stop=(i == KT - 1),
                                     perf_mode=DR)
                    nc.tensor.matmul(cpsB[:], vv[:, :, cb:cb + VSTR],
                                     epv[:, :, RQ:2 * RQ],
                                     start=(i == 0), stop=(i == KT - 1),
                                     perf_mode=DR)

                pend = None
                ep = None
                for kt in range(LT):
                    off = kt * 128
                    sc = scps.tile([128, 2 * RQ], f32, tag="sc", name="sc")
                    nc.tensor.matmul(sc[:, 0:RQ],
                                     kTr[hp][0:64, off:off + 128],
                                     qTr[hp][0:64, :], start=True, stop=True)
                    nc.tensor.matmul(sc[:, RQ:2 * RQ],
                                     kTr[hp][64:128, off:off + 128],
                                     qTr[hp][64:128, :], start=True, stop=True)
                    if kt % 2 == 0:
                        ep = poolE.tile([128, 4 * RQ], f8, tag="e", name="e")
                    nc.scalar.activation(
                        ep[:, (kt % 2) * 2 * RQ:(kt % 2 + 1) * 2 * RQ],
                        sc[:], AF.Exp, scale=0.125)
                    feed(1)
                    if kt % 2 == 1:
                        if pend is not None:
                            emit_ctx(*pend)
                        pend = (ep, kt // 2)
                    drain_to(v_pos[(half, min(kt + 1, LT - 1))])
                emit_ctx(*pend)
                nc.vector.tensor_copy(ctxraw[hA][:], cpsA[:])
                nc.vector.tensor_copy(ctxraw[hB][:], cpsB[:])
                s2 = poolCR.tile([2, RQ], bf16, tag="s2", name="s2")
                nc.sync.dma_start(s2[0:1, :], ctxraw[hA][64:65, :])
                nc.sync.dma_start(s2[1:2, :], ctxraw[hB][64:65, :])
                rec2 = poolCR.tile([2, RQ], f32, tag="rc", name="rc")
                nc.vector.reciprocal(rec2[:], s2[:])
                rcb = poolCR.tile([2, RQ], bf16, tag="rcb", name="rcb")
                nc.vector.tensor_copy(rcb[:], rec2[:])
                for h, cr in ((hA, ctxraw[hA]), (hB, ctxraw[hB])):
                    half2 = h % 2
                    rp = pjps.tile([128, 512], f32, tag="pj", name="pj")
                    nc.tensor.matmul(
                        rp[0:64, :],
                        onehot_sb[0:2, half2 * 64:half2 * 64 + 64],
                        rcb[:], start=True, stop=True)
                    dst = ctxTp[hp // 2][half2 * 64:half2 * 64 + 64,
                                         (hp % 2) * RQ:(hp % 2) * RQ + RQ]
                    nc.vector.tensor_mul(dst, cr[0:64, :], rp[0:64, :])

            stackP3.close()

        stackP2.close()

        # layer norm helper (takes its workspace pool)
        def layer_norm(dst, src, g_rep, be_rep, wpool):
            sview = src.rearrange("p (s d) -> p s d", s=2)
            stats = wpool.tile([128, 2, 6], f32, tag="lnstats",
                               name="lnstats")
            for sg in range(2):
                nc.vector.bn_stats(stats[:, sg, :], sview[:, sg, :])
            mv = wpool.tile([128, 2], f32, tag="lnmv", name="lnmv")
            nc.vector.bn_aggr(mv[:], stats[:])
            std = wpool.tile([128, 1], f32, tag="lnstd", name="lnstd")
            nc.scalar.activation(std[:], mv[:, 1:2], AF.Sqrt, bias=eps_sb[:])
            rstd = wpool.tile([128, 1], f32, tag="lnrstd", name="lnrstd")
            nc.vector.reciprocal(rstd[:], std[:])
            nc.vector.tensor_scalar(dst, src, mv[:, 0:1], rstd[:],
                                    op0=ALU.subtract, op1=ALU.mult)
            if g_rep is not None:
                nc.vector.tensor_mul(dst, dst, g_rep[:])
            if be_rep is not None:
                nc.vector.tensor_add(dst, dst, be_rep[:])

        # ---- w_o (fp8 DoubleRow) + residual + LN1 + transpose ----
        with tc.tile_pool(name="pwo", bufs=1) as poolWO, \
             tc.tile_pool(name="ph3w", bufs=2) as ph3w, \
             tc.tile_pool(name="aops", bufs=2, space="PSUM") as aops, \
             tc.tile_pool(name="tpps", bufs=4, space="PSUM") as tpps:
            wop = [poolWO.tile([128, 2 * D], f8, tag=f"wo{j}", name=f"wo{j}")
                   for j in range(KP)]
            xr_sb = [poolWO.tile([128, D], f32, tag=f"xr{t}", name=f"xr{t}")
                     for t in range(4)]
            for j in range(KP):
                ld2(wop[j][:, 0:D], wo[(2 * j) * 128:(2 * j + 1) * 128, :])
                ld2(wop[j][:, D:2 * D],
                    wo[(2 * j + 1) * 128:(2 * j + 2) * 128, :])
            for t in range(4):
                ld2(xr_sb[t][:], xr[t * 128:(t + 1) * 128, :])
            wov = [pairv(t) for t in wop]
            ctv = [pairv(t) for t in ctxTp]
            for qt in range(4):
                qsl = slice(qt * 128, qt * 128 + 128)
                ps = aops.tile([128, D], f32, tag="ao", name="ao")
                for half in range(2):
                    osl = slice(half * 512, half * 512 + 512)
                    for j in range(KP):
                        nc.tensor.matmul(ps[:, osl], ctv[j][:, :, qsl],
                                         wov[j][:, :, osl],
                                         start=(j == 0), stop=(j == KP - 1),
                                         perf_mode=DR)
                res = ph3w.tile([128, D], f32, tag="res", name="res")
                # fold the 1/64 w_o scale into the residual add
                nc.vector.scalar_tensor_tensor(
                    res[:], ps[:], 1.0 / 64.0, xr_sb[qt][:],
                    op0=ALU.mult, op1=ALU.add)
                if bo_rep is not None:
                    nc.vector.tensor_add(res[:], res[:], bo_rep[:])
                hq = ph3w.tile([128, D], f32, tag="hq", name="hq")
                layer_norm(hq[:], res[:], g1_rep, be1_rep, ph3w)
                nc.sync.dma_start(h_dram[qsl, :], hq[:])
                for m in range(KT):
                    tp = tpps.tile([128, 128], f32, tag="tp", name="tp")
                    nc.tensor.transpose(tp[:], hq[:, m * 128:m * 128 + 128],
                                        ident_sb[:])
                    nc.vector.tensor_copy(hT[m][:, qsl], tp[:])

        stackCtx.close()

        # ---- FFN (bf16; weights into all the freed space) ----
        with tc.tile_pool(name="pffn", bufs=1) as pffn, \
             tc.tile_pool(name="ph4w", bufs=2) as ph4w:
            ff1rT = [pffn.tile([128, RQ], bf16, tag=f"ff1{t}",
                               name=f"ff1{t}") for t in range(FT)]
            w1_sb = [pffn.tile([128, F], bf16, tag=f"w1{k}", name=f"w1{k}")
                     for k in range(KT)]
            w2_sb = [pffn.tile([128, D], bf16, tag=f"w2{k}", name=f"w2{k}")
                     for k in range(FT)]
            # w1 column-block-major so ffn1 chain ft=0 starts early
            for g in range(8):
                gsl = slice(g * 512, (g + 1) * 512)
                for k in range(KT):
                    ld2(w1_sb[k][:, gsl], w1[k * 128:(k + 1) * 128, gsl])
            # w2 half-column-major
            for half in range(2):
                osl = slice(half * 512, half * 512 + 512)
                for k in range(FT):
                    ld2(w2_sb[k][:, osl], w2[k * 128:(k + 1) * 128, osl])

            # ---- FFN up + ReLU ----
            with tc.tile_pool(name="f1ps", bufs=4, space="PSUM") as f1ps:
                for ft in range(FT):
                    fsl = slice(ft * 128, ft * 128 + 128)
                    ps = f1ps.tile([128, RQ], f32, tag="f1", name="f1")
                    for k in range(KT):
                        nc.tensor.matmul(ps[:], w1_sb[k][:, fsl], hT[k][:],
                                         start=(k == 0), stop=(k == KT - 1))
                    nc.scalar.activation(ff1rT[ft][:], ps[:], AF.Relu,
                                         bias=b1_sb[:, ft:ft + 1])

            # ---- FFN down + LN2 ----
            with tc.tile_pool(name="f2ps", bufs=3, space="PSUM") as f2ps:
                for qt in range(4):
                    qsl = slice(qt * 128, qt * 128 + 128)
                    ps = f2ps.tile([128, D], f32, tag="f2", name="f2")
                    for half in range(2):
                        osl = slice(half * 512, half * 512 + 512)
                        for ft in range(FT):
                            nc.tensor.matmul(ps[:, osl], ff1rT[ft][:, qsl],
                                             w2_sb[ft][:, osl],
                                             start=(ft == 0),
                                             stop=(ft == FT - 1))
                    hback = ph4w.tile([128, D], f32, tag="hback",
                                      name="hback")
                    nc.sync.dma_start(hback[:], h_dram[qsl, :])
                    res = ph4w.tile([128, D], f32, tag="res2", name="res2")
                    nc.vector.tensor_add(res[:], ps[:], hback[:])
                    if b2_rep is not None:
                        nc.vector.tensor_add(res[:], res[:], b2_rep[:])
                    o = ph4w.tile([128, D], f32, tag="out", name="out")
                    layer_norm(o[:], res[:], g2_rep, be2_rep, ph4w)
                    nc.sync.dma_start(y[qt * 128:(qt + 1) * 128, :], o[:])

    return nc


_CACHED = {}


def _get_program(flags):
    key = tuple(sorted(flags.items()))
    if key not in _CACHED:
        _CACHED[key] = _build_program(flags)
    return _CACHED[key]


def kernel(x, w_q, w_k, w_v, w_o, b_o, gamma1, beta1, gamma2, beta2,
           w1, b1, w2, b2, _trace=False):
    _install_patches()
    from concourse import bass_utils

    bf = ml_dtypes.bfloat16
    f8h = ml_dtypes.float8_e4m3
    x = np.asarray(x, np.float32)
    flags = {
        "use_bo": not np.all(np.asarray(b_o) == 0),
        "use_b2": not np.all(np.asarray(b2) == 0),
        "use_g1": not np.all(np.asarray(gamma1) == 1),
        "use_be1": not np.all(np.asarray(beta1) == 0),
        "use_g2": not np.all(np.asarray(gamma2) == 1),
        "use_be2": not np.all(np.asarray(beta2) == 0),
    }
    nc = _get_program(flags)

    # host-side shared prep. Head dims are permuted so the RoPE partner
    # (d <-> d+32) sits 16 partitions away within the same 32-partition
    # quadrant, making the partner swap a DVE stream_shuffle:
    #   new position p (within a 64-dim head) holds old dim PI[p].
    PI = np.concatenate([np.arange(0, 16), np.arange(32, 48),
                         np.arange(16, 32), np.arange(48, 64)])
    inv_freq = (1.0 / (10000.0 ** (np.arange(0, DK, 2, dtype=np.float64) / DK)))
    freqs = np.arange(L, dtype=np.float64)[:, None] * inv_freq      # [L, 32]
    cos = np.cos(freqs).T.astype(np.float32)                        # [32, L]
    sin = np.sin(freqs).T.astype(np.float32)
    fidx = PI % 32
    sgn = np.where(PI < 32, 1.0, -1.0).astype(np.float32)[:, None]
    # tables carry the 1/64 compensation for the x64 fp8 weight scale
    cos_rep = (np.concatenate([cos[fidx], cos[fidx]], 0) / 64.0).astype(bf)
    sin_sign = (np.concatenate([sgn * sin[fidx], sgn * sin[fidx]], 0)
                / 64.0).astype(bf)

    def permute_heads(w):
        wp = np.asarray(w, np.float32).reshape(D, H, DK)
        return np.ascontiguousarray(wp[:, :, PI].reshape(D, D))

    common = {
        "cosr": cos_rep, "sinr": sin_sign,
        "wq": (permute_heads(w_q) * 64).astype(f8h),
        "wk": (permute_heads(w_k) * 64).astype(f8h),
        "wv": (np.asarray(w_v, np.float32) * 32).astype(f8h),
        "wo": (np.asarray(w_o, np.float32) * 64).astype(f8h),
        "w1": w1.astype(bf), "w2": w2.astype(bf),
        "b1t": np.ascontiguousarray(
            np.asarray(b1, np.float32).reshape(F // 128, 128).T),
        "ident": np.eye(128, dtype=np.float32),
        "onehot": np.concatenate(
            [np.concatenate([np.ones((1, 64), np.float32),
                             np.zeros((1, 64), np.float32)], 1),
             np.concatenate([np.zeros((1, 64), np.float32),
                             np.ones((1, 64), np.float32)], 1)],
            0).astype(bf),
        "bo": np.asarray(b_o, np.float32).reshape(1, D),
        "b2r": np.asarray(b2, np.float32).reshape(1, D),
        "g1": np.asarray(gamma1, np.float32).reshape(1, D),
        "be1": np.asarray(beta1, np.float32).reshape(1, D),
        "g2": np.asarray(gamma2, np.float32).reshape(1, D),
        "be2": np.asarray(beta2, np.float32).reshape(1, D),
    }
    xT_all = [np.ascontiguousarray(x[b].T).astype(f8h) for b in range(B)]

    in_maps = []
    for c in range(NCORES):
        b, r = c // 4, c % 4
        rows = slice(r * RQ, (r + 1) * RQ)
        m = dict(common)
        m["xT"] = xT_all[b]
        m["xTq"] = np.ascontiguousarray(xT_all[b][:, rows])
        m["xr"] = np.ascontiguousarray(x[b, rows, :])
        m["qcos"] = np.ascontiguousarray(cos_rep[:, rows])
        m["qsin"] = np.ascontiguousarray(sin_sign[:, rows])
        in_maps.append(m)

    res = bass_utils.run_bass_kernel_spmd(
        nc, in_maps, core_ids=list(range(NCORES)), trace=_trace)

    out = np.empty((B, L, D), np.float32)
    for c in range(NCORES):
        b, r = c // 4, c % 4
        out[b, r * RQ:(r + 1) * RQ, :] = res.results[c]["y"]
    if _trace:
        kernel.last_exec_time_ns = res.exec_time_ns
    return out


# revision 16
# speedup vs baseline: 1.2183x; 1.0048x over previous
"""Trainium2 Bass kernel for a dense transformer encoder layer.

Shapes (hardcoded): B=2, L=2048, D=1024, F=4096, H=16 heads, dk=64.
Sharding over 8 NeuronCores: core c handles batch b=c//4 and query-row
quarter r=c%4 (512 rows). K/V projections for the full batch are
computed per core (replicated within the 4-core batch group; collectives
measured too slow here due to cross-core launch skew).

Performance structure:
- Projections are fused INTO the attention loop: the softmax exp stream
  (scalar engine) is attention's long pole, and K/V projection matmuls
  are emitted interleaved with score/context matmuls.
- Q/K/V/O projections and the attention context matmul run in fp8
  (e4m3) with DoubleRow perf mode (2 contraction tiles per matmul,
  ~1.4x tensor throughput). Weights are pre-scaled by 64 (w_v by 32)
  to stay in fp8 normal range; the scale is compensated for free in the
  RoPE tables (/64), the softmax ones-column (=32), and the w_o
  residual add (x 1/64). The FFN stays bf16 (fp8 there costs too much
  accuracy).
- RoPE's partner swap is a DVE stream_shuffle: head dims are permuted
  host-side so partners sit 16 partitions apart within one quadrant.
- Rescale is fused per head-pair; the epilogue (w_o + LN1 + FFN + LN2)
  is pipelined per query tile.
"""
import os
import sys
import types

sys.path.insert(0, "/opt/trn_rl_repo")

import numpy as np
import ml_dtypes

import concourse.bass as bass
import concourse.tile as tile
import concourse.mybir as mybir
from contextlib import ExitStack

f32 = mybir.dt.float32
bf16 = mybir.dt.bfloat16
f8 = mybir.dt.float8e4
AF = mybir.ActivationFunctionType
ALU = mybir.AluOpType
DR = mybir.MatmulPerfMode.DoubleRow

B, L, D, F, H, DK = 2, 2048, 1024, 4096, 16, 64
RQ = 512          # query rows per core
NCORES = 8
EPS = 1e-6
KT = D // 128      # 8 contraction tiles over D
KP = KT // 2       # 4 DoubleRow contraction pairs
NL = L // 512      # 4 column chunks over L
LT = L // 128      # 16 key tiles
FT = F // 128      # 32 f-tiles
VSTR = 65          # per-head stride in v tiles (64 v cols + ones)
VPAD = 528         # fp8 pair stride for v tiles (16-aligned)

_PATCHED = False


def _install_patches():
    """Register the NTFF profile hook (if available) and wrap the BIR
    compile step to split multi-wait instructions (this walrus build
    accepts at most one sync-wait per instruction)."""
    global _PATCHED
    if _PATCHED:
        return
    _PATCHED = True

    if "antenv.axon_hooks" not in sys.modules:
        try:
            from trn_agent_boot.trn_boot import _ntff_profile_via_ctypes
            hook = _ntff_profile_via_ctypes("/opt/axon/libaxon_pjrt.so")
        except Exception:
            hook = None
        mod = types.ModuleType("antenv.axon_hooks")
        mod.get_axon_ntff_profile_hook = lambda: hook
        mod.set_axon_ntff_profile_hook = lambda h: None
        sys.modules["antenv.axon_hooks"] = mod

    import json

    def _split_multiwaits(bir_bytes):
        d = json.loads(bir_bytes)
        ctr = 0
        for fn in d.get("functions", []):
            for blk in fn.get("blocks", []):
                out = []
                for inst in blk.get("instructions", []):
                    si = inst.get("sync_info")
                    ow = (si or {}).get("on_wait") or []
                    if len(ow) > 1 and inst.get("engine", "Unassigned") != "Unassigned":
                        for w in ow[:-1]:
                            out.append({
                                "debug": inst.get("debug", 0),
                                "engine": inst["engine"],
                                "ins": [], "outs": [],
                                "name": f"I-antsw{ctr}",
                                "opcode": "NoOp",
                                "sync_info": {"on_update": [], "on_wait": [w]},
                            })
                            ctr += 1
                        si["on_wait"] = [ow[-1]]
                    out.append(inst)
                blk["instructions"] = out
        return json.dumps(d).encode()

    import concourse.bass_utils as bu
    import concourse.bass2jax as b2j

    orig = bu.compile_bir_kernel

    def patched(bir_json, tmpdir, neff_name="file.neff"):
        return orig(_split_multiwaits(bir_json), tmpdir, neff_name=neff_name)

    bu.compile_bir_kernel = patched
    b2j.compile_bir_kernel = patched


def _build_program(flags):
    nc = bass.Bass("TRN2", target_bir_lowering=False, debug=False,
                   num_devices=NCORES)

    def din(name, shape, dt):
        return nc.dram_tensor(name, shape, dt, kind="ExternalInput").ap()

    xT = din("xT", [D, L], f8)              # batch x, transposed, fp8
    xTq = din("xTq", [D, RQ], f8)           # this core's columns of x[b].T
    xr = din("xr", [RQ, D], f32)            # this core's rows (residual)
    cosr = din("cosr", [128, L], bf16)      # cos table (permuted, /64)
    sinr = din("sinr", [128, L], bf16)      # sign-baked sin table (/64)
    qcos = din("qcos", [128, RQ], bf16)
    qsin = din("qsin", [128, RQ], bf16)
    wq = din("wq", [D, D], f8)              # x64, head-dims permuted
    wk = din("wk", [D, D], f8)              # x64, head-dims permuted
    wv = din("wv", [D, D], f8)              # x32
    wo = din("wo", [D, D], f8)              # x64
    w1 = din("w1", [D, F], bf16)
    w2 = din("w2", [F, D], bf16)
    b1t = din("b1t", [128, F // 128], f32)
    ident = din("ident", [128, 128], f32)
    onehot = din("onehot", [2, 128], bf16)
    bo = din("bo", [1, D], f32)
    b2r = din("b2r", [1, D], f32)
    g1 = din("g1", [1, D], f32)
    be1 = din("be1", [1, D], f32)
    g2 = din("g2", [1, D], f32)
    be2 = din("be2", [1, D], f32)
    y = nc.dram_tensor("y", [RQ, D], f32, kind="ExternalOutput").ap()

    def bcast_ap(ap2d, width):
        return bass.AP(tensor=ap2d.tensor, offset=ap2d.offset,
                       ap=[[0, 128], [1, width]])

    def pairv(t):
        # [128, 2*W] tile -> [128, 2, W] DoubleRow view
        return t[:].rearrange("p (a b) -> p a b", a=2)

    with tile.TileContext(nc) as tc:
      with ExitStack() as top:
        consts = top.enter_context(tc.tile_pool(name="consts", bufs=1))
        poolP1 = top.enter_context(tc.tile_pool(name="p1", bufs=1))
        dramp = top.enter_context(tc.tile_pool(name="dramp", bufs=1,
                                               space="DRAM"))
        h_dram = dramp.tile([RQ, D], f32, tag="h_dram", name="h_dram")
        stackCtx = ExitStack()
        poolCtx = stackCtx.enter_context(tc.tile_pool(name="pctx", bufs=1))
        stackP2 = ExitStack()
        poolP2 = stackP2.enter_context(tc.tile_pool(name="p2", bufs=1))
        poolE = stackP2.enter_context(tc.tile_pool(name="pe", bufs=3))
        poolCR = stackP2.enter_context(tc.tile_pool(name="pcr", bufs=2))
        stackP3 = ExitStack()
        poolP3 = stackP3.enter_context(tc.tile_pool(name="p3", bufs=1))
        poolRW = stackP3.enter_context(tc.tile_pool(name="prw", bufs=2))

        _qs = [nc.sync, nc.scalar, nc.gpsimd]
        _qi = [0]

        def ld(dst, src):
            _qs[_qi[0] % 3].dma_start(dst, src)
            _qi[0] += 1

        _qs2 = [nc.sync, nc.gpsimd]
        _qi2 = [0]

        def ld2(dst, src):
            # loads issued while the exp stream owns the scalar queue
            _qs2[_qi2[0] % 2].dma_start(dst, src)
            _qi2[0] += 1

        # --- long-lived constants ---
        ident_sb = consts.tile([128, 128], f32, tag="ident", name="ident")
        nc.sync.dma_start(ident_sb[:], ident[:])
        b1_sb = consts.tile([128, F // 128], f32, tag="b1", name="b1")
        nc.scalar.dma_start(b1_sb[:], b1t[:])
        onehot_sb = consts.tile([2, 128], bf16, tag="onehot", name="onehot")
        nc.gpsimd.dma_start(onehot_sb[:], onehot[:])
        eps_sb = consts.tile([128, 1], f32, tag="eps", name="eps")
        nc.vector.memset(eps_sb[:], EPS)

        def rep_const(ap2d, use, tag):
            if not use:
                return None
            t = consts.tile([128, D], f32, tag=tag, name=tag)
            nc.gpsimd.dma_start(out=t[:], in_=bcast_ap(ap2d, D))
            return t

        bo_rep = rep_const(bo, flags["use_bo"], "bo")
        b2_rep = rep_const(b2r, flags["use_b2"], "b2")
        g1_rep = rep_const(g1, flags["use_g1"], "g1")
        be1_rep = rep_const(be1, flags["use_be1"], "be1")
        g2_rep = rep_const(g2, flags["use_g2"], "g2")
        be2_rep = rep_const(be2, flags["use_be2"], "be2")

        # hT persists from the w_o phase into the FFN (top-level pool)
        hT = [poolP1.tile([128, RQ], bf16, tag=f"hT{k}", name=f"hT{k}")
              for k in range(KT)]
        # preloaded epilogue weights (loads run during attention)
        wop = [poolP1.tile([128, 2 * D], f8, tag=f"wo{j}", name=f"wo{j}")
               for j in range(KP)]
        w1a = [poolP1.tile([128, F // 2], bf16, tag=f"w1a{k}",
                           name=f"w1a{k}") for k in range(KT)]
        # ctxT pairs (fp8, DoubleRow stationary for w_o)
        ctxTp = [poolCtx.tile([128, 2 * RQ], f8, tag=f"ctxT{j}",
                              name=f"ctxT{j}") for j in range(KP)]

        # attention-lived arrays
        kTr = [poolP2.tile([128, L], bf16, tag=f"kTr{m}", name=f"kTr{m}")
               for m in range(KT)]
        qTr = [poolP2.tile([128, RQ], bf16, tag=f"qTr{m}", name=f"qTr{m}")
               for m in range(KT)]
        # v pairs (fp8, DoubleRow stationary for ctx); halves at 0/VPAD
        vpl = [poolP2.tile([128, 2 * VPAD], f8, tag=f"vl{i}", name=f"vl{i}")
               for i in range(KT)]
        vph = [poolP2.tile([128, 2 * VPAD], f8, tag=f"vh{i}", name=f"vh{i}")
               for i in range(KT)]
        ctxraw = [poolP2.tile([VSTR, RQ], bf16, tag=f"cr{h}", name=f"cr{h}")
                  for h in range(H)]

        # projection inputs (fp8 pairs for DoubleRow)
        xtp = [poolP3.tile([128, 2 * L], f8, tag=f"xt{j}", name=f"xt{j}")
               for j in range(KP)]
        xqp = [poolP3.tile([128, 2 * RQ], f8, tag=f"xq{j}", name=f"xq{j}")
               for j in range(KP)]
        wkp = [poolP3.tile([128, 2 * D], f8, tag=f"wk{j}", name=f"wk{j}")
               for j in range(KP)]
        wqp = [poolP3.tile([128, 2 * D], f8, tag=f"wq{j}", name=f"wq{j}")
               for j in range(KP)]
        wvp = [poolP3.tile([128, 2 * D], f8, tag=f"wv{j}", name=f"wv{j}")
               for j in range(KP)]
        cos_sb = poolP3.tile([128, L], bf16, tag="cos", name="cos")
        sin_sb = poolP3.tile([128, L], bf16, tag="sin", name="sin")
        qcos_sb = poolP3.tile([128, RQ], bf16, tag="qcos", name="qcos")
        qsin_sb = poolP3.tile([128, RQ], bf16, tag="qsin", name="qsin")

        def ld_pair(dsts, src, width):
            for j in range(KP):
                ld(dsts[j][:, 0:width], src[(2 * j) * 128:(2 * j + 1) * 128, :])
                ld(dsts[j][:, width:2 * width],
                   src[(2 * j + 1) * 128:(2 * j + 2) * 128, :])

        # load order: k-proj inputs first (they gate the exp stream)
        ld_pair(xtp, xT, L)
        ld_pair(wkp, wk, D)
        ld_pair(wqp, wq, D)
        ld_pair(xqp, xTq, RQ)
        ld(cos_sb[:], cosr[:])
        ld(sin_sb[:], sinr[:])
        ld(qcos_sb[:], qcos[:])
        ld(qsin_sb[:], qsin[:])
        ld_pair(wvp, wv, D)
        for j in range(KP):
            ld(wop[j][:, 0:D], wo[(2 * j) * 128:(2 * j + 1) * 128, :])
            ld(wop[j][:, D:2 * D],
               wo[(2 * j + 1) * 128:(2 * j + 2) * 128, :])
        for g in range(4):
            gsl = slice(g * 512, (g + 1) * 512)
            for k in range(KT):
                ld(w1a[k][:, gsl], w1[k * 128:(k + 1) * 128, gsl])

        xtv = [pairv(t) for t in xtp]
        xqv = [pairv(t) for t in xqp]
        wkv = [pairv(t) for t in wkp]
        wqv = [pairv(t) for t in wqp]
        wvv = [pairv(t) for t in wvp]

        def rope_chunk(ps, cos_sl, sin_sl, dst):
            """dst = ps*cos + shuffle16(ps*sin); tables carry the 1/64."""
            tct = poolRW.tile([128, 512], bf16, tag="rtc", name="rtc")
            nc.vector.tensor_mul(tct[:], ps, cos_sl)
            tsn = poolRW.tile([128, 512], bf16, tag="rtm", name="rtm")
            nc.vector.tensor_mul(tsn[:], ps, sin_sl)
            tsw = poolRW.tile([128, 512], bf16, tag="tsw", name="tsw")
            nc.vector.stream_shuffle(tsw[:], tsn[:],
                                     [(i + 16) % 32 for i in range(32)])
            nc.vector.tensor_add(dst, tct[:], tsw[:])

        with tc.tile_pool(name="pjps", bufs=2, space="PSUM") as pjps, \
             tc.tile_pool(name="scps", bufs=2, space="PSUM") as scps, \
             tc.tile_pool(name="ctxps", bufs=1, space="PSUM") as ctxps:

            def q_proj(m):
                msl = slice(m * 128, m * 128 + 128)
                ps = pjps.tile([128, 512], f32, tag="pj", name="pj")
                for j in range(KP):
                    nc.tensor.matmul(ps[:], wqv[j][:, :, msl], xqv[j][:],
                                     start=(j == 0), stop=(j == KP - 1),
                                     perf_mode=DR)
                rope_chunk(ps[:], qcos_sb[:], qsin_sb[:], qTr[m][:])

            def k_proj(m, n):
                msl = slice(m * 128, m * 128 + 128)
                nsl = slice(n * 512, n * 512 + 512)
                ps = pjps.tile([128, 512], f32, tag="pj", name="pj")
                for j in range(KP):
                    nc.tensor.matmul(ps[:], wkv[j][:, :, msl],
                                     xtv[j][:, :, nsl],
                                     start=(j == 0), stop=(j == KP - 1),
                                     perf_mode=DR)
                rope_chunk(ps[:], cos_sb[:, nsl], sin_sb[:, nsl],
                           kTr[m][:, nsl])

            # kTr[0] first (gates the exp stream), then q projection
            for n in range(NL):
                k_proj(0, n)
            for m in range(KT):
                q_proj(m)

            def v_proj(half, t):
                tsl = slice(t * 128, t * 128 + 128)
                ps = pjps.tile([128, 512], f32, tag="pj", name="pj")
                for j in range(KP):
                    nc.tensor.matmul(ps[:], xtv[j][:, :, tsl],
                                     wvv[j][:, :, half * 512:half * 512 + 512],
                                     start=(j == 0), stop=(j == KP - 1),
                                     perf_mode=DR)
                vp = (vpl if half == 0 else vph)[t // 2]
                off = (t % 2) * VPAD
                vview = vp[:, off:off + 8 * VSTR].rearrange(
                    "p (h e) -> p h e", h=8)
                ps_view = ps[:].rearrange("p (h e) -> p h e", h=8)
                nc.vector.tensor_copy(vview[:, :, 0:DK], ps_view[:])
                # ones column carries the 1/32 compensation for wv's x32
                nc.vector.memset(vview[:, :, DK:DK + 1], 32.0)

            # deferred projection work, emitted interleaved into attention
            stream = []
            for t in range(LT):
                stream.append(("v0", 0, t))
            for m in range(1, 5):
                for n in range(NL):
                    stream.append(("k", m, n))
            for t in range(LT):
                stream.append(("v1", 1, t))
            for m in range(5, KT):
                for n in range(NL):
                    stream.append(("k", m, n))
            v_pos = {}
            k_pos = {}
            for i, it in enumerate(stream):
                if it[0] == "k":
                    k_pos[it[1]] = i
                else:
                    v_pos[(it[1], it[2])] = i
            fed = [0]

            def emit(it):
                if it[0] == "k":
                    k_proj(it[1], it[2])
                else:
                    v_proj(it[1], it[2])

            def feed(n):
                for _ in range(n):
                    if fed[0] < len(stream):
                        emit(stream[fed[0]])
                        fed[0] += 1

            def drain_to(idx):
                while fed[0] <= idx:
                    emit(stream[fed[0]])
                    fed[0] += 1

            # ---- attention (projection stream fed between steps) ----
            for hp in range(KT):
                if hp >= 1:
                    drain_to(k_pos[hp])
                hA, hB = 2 * hp, 2 * hp + 1
                half = 0 if hp < 4 else 1
                vt = vpl if hp < 4 else vph
                ca = (hA % 8) * VSTR
                cb = (hB % 8) * VSTR
                cpsA = ctxps.tile([VSTR, RQ], f32, tag="cpsA", name="cpsA")
                cpsB = ctxps.tile([VSTR, RQ], f32, tag="cpsB", name="cpsB")

                def emit_ctx(ep, i):
                    epv = pairv(ep)
                    vv = pairv(vt[i])
                    nc.tensor.matmul(cpsA[:], vv[:, :, ca:ca + VSTR],
                                     epv[:, :, 0:RQ],
                                     start=(i == 0), stop=(i == KT - 1),
                                     perf_mode=DR)
                    nc.tensor.matmul(cpsB[:], vv[:, :, cb:cb + VSTR],
                                     epv[:, :, RQ:2 * RQ],
                                     start=(i == 0), stop=(i == KT - 1),
                                     perf_mode=DR)

                pend = None
                ep = None
                for kt in range(LT):
                    off = kt * 128
                    sc = scps.tile([128, 2 * RQ], f32, tag="sc", name="sc")
                    nc.tensor.matmul(sc[:, 0:RQ],
                                     kTr[hp][0:64, off:off + 128],
                                     qTr[hp][0:64, :], start=True, stop=True)
                    nc.tensor.matmul(sc[:, RQ:2 * RQ],
                                     kTr[hp][64:128, off:off + 128],
                                     qTr[hp][64:128, :], start=True, stop=True)
                    if kt % 2 == 0:
                        ep = poolE.tile([128, 4 * RQ], f8, tag="e", name="e")
                    nc.scalar.activation(
                        ep[:, (kt % 2) * 2 * RQ:(kt % 2 + 1) * 2 * RQ],
                        sc[:], AF.Exp, scale=0.125)
                    feed(1)
                    if kt % 2 == 1:
                        if pend is not None:
                            emit_ctx(*pend)
                        pend = (ep, kt // 2)
                    drain_to(v_pos[(half, min(kt + 1, LT - 1))])
                emit_ctx(*pend)
                nc.vector.tensor_copy(ctxraw[hA][:], cpsA[:])
                nc.vector.tensor_copy(ctxraw[hB][:], cpsB[:])
                s2 = poolCR.tile([2, RQ], bf16, tag="s2", name="s2")
                nc.sync.dma_start(s2[0:1, :], ctxraw[hA][64:65, :])
                nc.sync.dma_start(s2[1:2, :], ctxraw[hB][64:65, :])
                rec2 = poolCR.tile([2, RQ], f32, tag="rc", name="rc")
                nc.vector.reciprocal(rec2[:], s2[:])
                rcb = poolCR.tile([2, RQ], bf16, tag="rcb", name="rcb")
                nc.vector.tensor_copy(rcb[:], rec2[:])
                for h, cr in ((hA, ctxraw[hA]), (hB, ctxraw[hB])):
                    half2 = h % 2
                    rp = pjps.tile([128, 512], f32, tag="pj", name="pj")
                    nc.tensor.matmul(
                        rp[0:64, :],
                        onehot_sb[0:2, half2 * 64:half2 * 64 + 64],
                        rcb[:], start=True, stop=True)
                    dst = ctxTp[hp // 2][half2 * 64:half2 * 64 + 64,
                                         (hp % 2) * RQ:(hp % 2) * RQ + RQ]
                    nc.vector.tensor_mul(dst, cr[0:64, :], rp[0:64, :])

            stackP3.close()

        stackP2.close()

        # layer norm helper (takes its workspace pool)
        def layer_norm(dst, src, g_rep, be_rep, wpool):
            sview = src.rearrange("p (s d) -> p s d", s=2)
            stats = wpool.tile([128, 2, 6], f32, tag="lnstats",
                               name="lnstats")
            for sg in range(2):
                nc.vector.bn_stats(stats[:, sg, :], sview[:, sg, :])
            mv = wpool.tile([128, 2], f32, tag="lnmv", name="lnmv")
            nc.vector.bn_aggr(mv[:], stats[:])
            std = wpool.tile([128, 1], f32, tag="lnstd", name="lnstd")
            nc.scalar.activation(std[:], mv[:, 1:2], AF.Sqrt, bias=eps_sb[:])
            rstd = wpool.tile([128, 1], f32, tag="lnrstd", name="lnrstd")
            nc.vector.reciprocal(rstd[:], std[:])
            nc.vector.tensor_scalar(dst, src, mv[:, 0:1], rstd[:],
                                    op0=ALU.subtract, op1=ALU.mult)
            if g_rep is not None:
                nc.vector.tensor_mul(dst, dst, g_rep[:])
            if be_rep is not None:
                nc.vector.tensor_add(dst, dst, be_rep[:])

        # ---- w_o (fp8 DoubleRow) + residual + LN1 + transpose ----
        with tc.tile_pool(name="pwo", bufs=1) as poolWO, \
             tc.tile_pool(name="ph3w", bufs=2) as ph3w, \
             tc.tile_pool(name="aops", bufs=2, space="PSUM") as aops, \
             tc.tile_pool(name="tpps", bufs=4, space="PSUM") as tpps:
            xr_sb = [poolWO.tile([128, D], f32, tag=f"xr{t}", name=f"xr{t}")
                     for t in range(4)]
            for t in range(4):
                ld2(xr_sb[t][:], xr[t * 128:(t + 1) * 128, :])
            wov = [pairv(t) for t in wop]
            ctv = [pairv(t) for t in ctxTp]
            for qt in range(4):
                qsl = slice(qt * 128, qt * 128 + 128)
                ps = aops.tile([128, D], f32, tag="ao", name="ao")
                for half in range(2):
                    osl = slice(half * 512, half * 512 + 512)
                    for j in range(KP):
                        nc.tensor.matmul(ps[:, osl], ctv[j][:, :, qsl],
                                         wov[j][:, :, osl],
                                         start=(j == 0), stop=(j == KP - 1),
                                         perf_mode=DR)
                res = ph3w.tile([128, D], f32, tag="res", name="res")
                # fold the 1/64 w_o scale into the residual add
                nc.vector.scalar_tensor_tensor(
                    res[:], ps[:], 1.0 / 64.0, xr_sb[qt][:],
                    op0=ALU.mult, op1=ALU.add)
                if bo_rep is not None:
                    nc.vector.tensor_add(res[:], res[:], bo_rep[:])
                hq = ph3w.tile([128, D], f32, tag="hq", name="hq")
                layer_norm(hq[:], res[:], g1_rep, be1_rep, ph3w)
                nc.sync.dma_start(h_dram[qsl, :], hq[:])
                for m in range(KT):
                    tp = tpps.tile([128, 128], f32, tag="tp", name="tp")
                    nc.tensor.transpose(tp[:], hq[:, m * 128:m * 128 + 128],
                                        ident_sb[:])
                    nc.vector.tensor_copy(hT[m][:, qsl], tp[:])

        stackCtx.close()

        # ---- FFN (bf16; weights into all the freed space) ----
        with tc.tile_pool(name="pffn", bufs=1) as pffn, \
             tc.tile_pool(name="ph4w", bufs=2) as ph4w:
            ff1rT = [pffn.tile([128, RQ], bf16, tag=f"ff1{t}",
                               name=f"ff1{t}") for t in range(FT)]
            w1b = [pffn.tile([128, F // 2], bf16, tag=f"w1b{k}",
                             name=f"w1b{k}") for k in range(KT)]
            w2_sb = [pffn.tile([128, D], bf16, tag=f"w2{k}", name=f"w2{k}")
                     for k in range(FT)]
            # second w1 column half (first half preloaded during attention)
            for g in range(4):
                gsl = slice(g * 512, (g + 1) * 512)
                for k in range(KT):
                    ld2(w1b[k][:, gsl],
                        w1[k * 128:(k + 1) * 128, F // 2 + g * 512:
                           F // 2 + (g + 1) * 512])
            # w2 half-column-major
            for half in range(2):
                osl = slice(half * 512, half * 512 + 512)
                for k in range(FT):
                    ld2(w2_sb[k][:, osl], w2[k * 128:(k + 1) * 128, osl])

            # ---- FFN up + ReLU ----
            with tc.tile_pool(name="f1ps", bufs=4, space="PSUM") as f1ps:
                for ft in range(FT):
                    w1t = w1a if ft < FT // 2 else w1b
                    fo = (ft % (FT // 2)) * 128
                    ps = f1ps.tile([128, RQ], f32, tag="f1", name="f1")
                    for k in range(KT):
                        nc.tensor.matmul(ps[:], w1t[k][:, fo:fo + 128],
                                         hT[k][:],
                                         start=(k == 0), stop=(k == KT - 1))
                    nc.scalar.activation(ff1rT[ft][:], ps[:], AF.Relu,
                                         bias=b1_sb[:, ft:ft + 1])

            # ---- FFN down + LN2 ----
            with tc.tile_pool(name="f2ps", bufs=3, space="PSUM") as f2ps:
                for qt in range(4):
                    qsl = slice(qt * 128, qt * 128 + 128)
                    ps = f2ps.tile([128, D], f32, tag="f2", name="f2")
                    for half in range(2):
                        osl = slice(half * 512, half * 512 + 512)
                        for ft in range(FT):
                            nc.tensor.matmul(ps[:, osl], ff1rT[ft][:, qsl],
                                             w2_sb[ft][:, osl],
                                             start=(ft == 0),
                                             stop=(ft == FT - 1))
                    hback = ph4w.tile([128, D], f32, tag="hback",
                                      name="hback")
                    nc.sync.dma_start(hback[:], h_dram[qsl, :])
                    res = ph4w.tile([128, D], f32, tag="res2", name="res2")
                    nc.vector.tensor_add(res[:], ps[:], hback[:])
                    if b2_rep is not None:
                        nc.vector.tensor_add(res[:], res[:], b2_rep[:])
                    o = ph4w.tile([128, D], f32, tag="out", name="out")
                    layer_norm(o[:], res[:], g2_rep, be2_rep, ph4w)
                    nc.sync.dma_start(y[qt * 128:(qt + 1) * 128, :], o[:])

    return nc


_CACHED = {}


def _get_program(flags):
    key = tuple(sorted(flags.items()))
    if key not in _CACHED:
        _CACHED[key] = _build_program(flags)
    return _CACHED[key]


def kernel(x, w_q, w_k, w_v, w_o, b_o, gamma1, beta1, gamma2, beta2,
           w1, b1, w2, b2, _trace=False):
    _install_patches()
    from concourse import bass_utils

    bf = ml_dtypes.bfloat16
    f8h = ml_dtypes.float8_e4m3
    x = np.asarray(x, np.float32)
    flags = {
        "use_bo": not np.all(np.asarray(b_o) == 0),
        "use_b2": not np.all(np.asarray(b2) == 0),
        "use_g1": not np.all(np.asarray(gamma1) == 1),
        "use_be1": not np.all(np.asarray(beta1) == 0),
        "use_g2": not np.all(np.asarray(gamma2) == 1),
        "use_be2": not np.all(np.asarray(beta2) == 0),
    }
    nc = _get_program(flags)

    # host-side shared prep. Head dims are permuted so the RoPE partner
    # (d <-> d+32) sits 16 partitions away within the same 32-partition
    # quadrant, making the partner swap a DVE stream_shuffle:
    #   new position p (within a 64-dim head) holds old dim PI[p].
    PI = np.concatenate([np.arange(0, 16), np.arange(32, 48),
                         np.arange(16, 32), np.arange(48, 64)])
    inv_freq = (1.0 / (10000.0 ** (np.arange(0, DK, 2, dtype=np.float64) / DK)))
    freqs = np.arange(L, dtype=np.float64)[:, None] * inv_freq      # [L, 32]
    cos = np.cos(freqs).T.astype(np.float32)                        # [32, L]
    sin = np.sin(freqs).T.astype(np.float32)
    fidx = PI % 32
    sgn = np.where(PI < 32, 1.0, -1.0).astype(np.float32)[:, None]
    # tables carry the 1/64 compensation for the x64 fp8 weight scale
    cos_rep = (np.concatenate([cos[fidx], cos[fidx]], 0) / 64.0).astype(bf)
    sin_sign = (np.concatenate([sgn * sin[fidx], sgn * sin[fidx]], 0)
                / 64.0).astype(bf)

    def permute_heads(w):
        wp = np.asarray(w, np.float32).reshape(D, H, DK)
        return np.ascontiguousarray(wp[:, :, PI].reshape(D, D))

    common = {
        "cosr": cos_rep, "sinr": sin_sign,
        "wq": (permute_heads(w_q) * 64).astype(f8h),
        "wk": (permute_heads(w_k) * 64).astype(f8h),
        "wv": (np.asarray(w_v, np.float32) * 32).astype(f8h),
        "wo": (np.asarray(w_o, np.float32) * 64).astype(f8h),
        "w1": w1.astype(bf), "w2": w2.astype(bf),
        "b1t": np.ascontiguousarray(
            np.asarray(b1, np.float32).reshape(F // 128, 128).T),
        "ident": np.eye(128, dtype=np.float32),
        "onehot": np.concatenate(
            [np.concatenate([np.ones((1, 64), np.float32),
                             np.zeros((1, 64), np.float32)], 1),
             np.concatenate([np.zeros((1, 64), np.float32),
                             np.ones((1, 64), np.float32)], 1)],
            0).astype(bf),
        "bo": np.asarray(b_o, np.float32).reshape(1, D),
        "b2r": np.asarray(b2, np.float32).reshape(1, D),
        "g1": np.asarray(gamma1, np.float32).reshape(1, D),
        "be1": np.asarray(beta1, np.float32).reshape(1, D),
        "g2": np.asarray(gamma2, np.float32).reshape(1, D),
        "be2": np.asarray(beta2, np.float32).reshape(1, D),
    }
    xT_all = [np.ascontiguousarray(x[b].T).astype(f8h) for b in range(B)]

    in_maps = []
    for c in range(NCORES):
        b, r = c // 4, c % 4
        rows = slice(r * RQ, (r + 1) * RQ)
        m = dict(common)
        m["xT"] = xT_all[b]
        m["xTq"] = np.ascontiguousarray(xT_all[b][:, rows])
        m["xr"] = np.ascontiguousarray(x[b, rows, :])
        m["qcos"] = np.ascontiguousarray(cos_rep[:, rows])
        m["qsin"] = np.ascontiguousarray(sin_sign[:, rows])
        in_maps.append(m)

    res = bass_utils.run_bass_kernel_spmd(
        nc, in_maps, core_ids=list(range(NCORES)), trace=_trace)

    out = np.empty((B, L, D), np.float32)
    for c in range(NCORES):
        b, r = c // 4, c % 4
        out[b, r * RQ:(r + 1) * RQ, :] = res.results[c]["y"]
    if _trace:
        kernel.last_exec_time_ns = res.exec_time_ns
    return out


# revision 17
# speedup vs baseline: 1.2216x; 1.0027x over previous
"""Trainium2 Bass kernel for a dense transformer encoder layer.

Shapes (hardcoded): B=2, L=2048, D=1024, F=4096, H=16 heads, dk=64.
Sharding over 8 NeuronCores: core c handles batch b=c//4 and query-row
quarter r=c%4 (512 rows). K/V projections for the full batch are
computed per core (replicated within the 4-core batch group; collectives
measured too slow here due to cross-core launch skew).

Performance structure:
- Projections are fused INTO the attention loop: the softmax exp stream
  (scalar engine) is attention's long pole, and K/V projection matmuls
  are emitted interleaved with score/context matmuls.
- Q/K/V/O projections and the attention context matmul run in fp8
  (e4m3) with DoubleRow perf mode (2 contraction tiles per matmul,
  ~1.4x tensor throughput). Weights are pre-scaled by 64 (w_v by 32)
  to stay in fp8 normal range; the scale is compensated for free in the
  RoPE tables (/64), the softmax ones-column (=32), and the w_o
  residual add (x 1/64). The FFN stays bf16 (fp8 there costs too much
  accuracy).
- RoPE's partner swap is a DVE stream_shuffle: head dims are permuted
  host-side so partners sit 16 partitions apart within one quadrant.
- Rescale is fused per head-pair; the epilogue (w_o + LN1 + FFN + LN2)
  is pipelined per query tile.
"""
import os
import sys
import types

sys.path.insert(0, "/opt/trn_rl_repo")

import numpy as np
import ml_dtypes

import concourse.bass as bass
import concourse.tile as tile
import concourse.mybir as mybir
from contextlib import ExitStack

f32 = mybir.dt.float32
bf16 = mybir.dt.bfloat16
f8 = mybir.dt.float8e4
AF = mybir.ActivationFunctionType
ALU = mybir.AluOpType
DR = mybir.MatmulPerfMode.DoubleRow

B, L, D, F, H, DK = 2, 2048, 1024, 4096, 16, 64
RQ = 512          # query rows per core
NCORES = 8
EPS = 1e-6
KT = D // 128      # 8 contraction tiles over D
KP = KT // 2       # 4 DoubleRow contraction pairs
NL = L // 512      # 4 column chunks over L
LT = L // 128      # 16 key tiles
FT = F // 128      # 32 f-tiles
VSTR = 65          # per-head stride in v tiles (64 v cols + ones)
VPAD = 528         # fp8 pair stride for v tiles (16-aligned)

_PATCHED = False


def _install_patches():
    """Register the NTFF profile hook (if available) and wrap the BIR
    compile step to split multi-wait instructions (this walrus build
    accepts at most one sync-wait per instruction)."""
    global _PATCHED
    if _PATCHED:
        return
    _PATCHED = True

    if "antenv.axon_hooks" not in sys.modules:
        try:
            from trn_agent_boot.trn_boot import _ntff_profile_via_ctypes
            hook = _ntff_profile_via_ctypes("/opt/axon/libaxon_pjrt.so")
        except Exception:
            hook = None
        mod = types.ModuleType("antenv.axon_hooks")
        mod.get_axon_ntff_profile_hook = lambda: hook
        mod.set_axon_ntff_profile_hook = lambda h: None
        sys.modules["antenv.axon_hooks"] = mod

    import json

    def _split_multiwaits(bir_bytes):
        d = json.loads(bir_bytes)
        ctr = 0
        for fn in d.get("functions", []):
            for blk in fn.get("blocks", []):
                out = []
                for inst in blk.get("instructions", []):
                    si = inst.get("sync_info")
                    ow = (si or {}).get("on_wait") or []
                    if len(ow) > 1 and inst.get("engine", "Unassigned") != "Unassigned":
                        for w in ow[:-1]:
                            out.append({
                                "debug": inst.get("debug", 0),
                                "engine": inst["engine"],
                                "ins": [], "outs": [],
                                "name": f"I-antsw{ctr}",
                                "opcode": "NoOp",
                                "sync_info": {"on_update": [], "on_wait": [w]},
                            })
                            ctr += 1
                        si["on_wait"] = [ow[-1]]
                    out.append(inst)
                blk["instructions"] = out
        return json.dumps(d).encode()

    import concourse.bass_utils as bu
    import concourse.bass2jax as b2j

    orig = bu.compile_bir_kernel

    def patched(bir_json, tmpdir, neff_name="file.neff"):
        return orig(_split_multiwaits(bir_json), tmpdir, neff_name=neff_name)

    bu.compile_bir_kernel = patched
    b2j.compile_bir_kernel = patched


def _build_program(flags):
    nc = bass.Bass("TRN2", target_bir_lowering=False, debug=False,
                   num_devices=NCORES)

    def din(name, shape, dt):
        return nc.dram_tensor(name, shape, dt, kind="ExternalInput").ap()

    xT = din("xT", [D, L], f8)              # batch x, transposed, fp8
    xTq = din("xTq", [D, RQ], f8)           # this core's columns of x[b].T
    xr = din("xr", [RQ, D], f32)            # this core's rows (residual)
    cosr = din("cosr", [128, L], bf16)      # cos table (permuted, /64)
    sinr = din("sinr", [128, L], bf16)      # sign-baked sin table (/64)
    qcos = din("qcos", [128, RQ], bf16)
    qsin = din("qsin", [128, RQ], bf16)
    wq = din("wq", [D, D], f8)              # x64, head-dims permuted
    wk = din("wk", [D, D], f8)              # x64, head-dims permuted
    wv = din("wv", [D, D], f8)              # x32
    wo = din("wo", [D, D], f8)              # x64
    w1 = din("w1", [D, F], bf16)
    w2 = din("w2", [F, D], bf16)
    b1t = din("b1t", [128, F // 128], f32)
    ident = din("ident", [128, 128], f32)
    onehot = din("onehot", [2, 128], bf16)
    bo = din("bo", [1, D], f32)
    b2r = din("b2r", [1, D], f32)
    g1 = din("g1", [1, D], f32)
    be1 = din("be1", [1, D], f32)
    g2 = din("g2", [1, D], f32)
    be2 = din("be2", [1, D], f32)
    y = nc.dram_tensor("y", [RQ, D], f32, kind="ExternalOutput").ap()

    def bcast_ap(ap2d, width):
        return bass.AP(tensor=ap2d.tensor, offset=ap2d.offset,
                       ap=[[0, 128], [1, width]])

    def pairv(t):
        # [128, 2*W] tile -> [128, 2, W] DoubleRow view
        return t[:].rearrange("p (a b) -> p a b", a=2)

    with tile.TileContext(nc) as tc:
      with ExitStack() as top:
        consts = top.enter_context(tc.tile_pool(name="consts", bufs=1))
        poolP1 = top.enter_context(tc.tile_pool(name="p1", bufs=1))
        dramp = top.enter_context(tc.tile_pool(name="dramp", bufs=1,
                                               space="DRAM"))
        h_dram = dramp.tile([RQ, D], f32, tag="h_dram", name="h_dram")
        stackCtx = ExitStack()
        poolCtx = stackCtx.enter_context(tc.tile_pool(name="pctx", bufs=1))
        stackP2 = ExitStack()
        poolP2 = stackP2.enter_context(tc.tile_pool(name="p2", bufs=1))
        poolE = stackP2.enter_context(tc.tile_pool(name="pe", bufs=3))
        poolCR = stackP2.enter_context(tc.tile_pool(name="pcr", bufs=2))
        stackP3 = ExitStack()
        poolP3 = stackP3.enter_context(tc.tile_pool(name="p3", bufs=1))
        poolRW = stackP3.enter_context(tc.tile_pool(name="prw", bufs=2))

        _qs = [nc.sync, nc.scalar, nc.gpsimd]
        _qi = [0]

        def ld(dst, src):
            _qs[_qi[0] % 3].dma_start(dst, src)
            _qi[0] += 1

        _qs2 = [nc.sync, nc.gpsimd]
        _qi2 = [0]

        def ld2(dst, src):
            # loads issued while the exp stream owns the scalar queue
            _qs2[_qi2[0] % 2].dma_start(dst, src)
            _qi2[0] += 1

        # --- long-lived constants ---
        ident_sb = consts.tile([128, 128], f32, tag="ident", name="ident")
        nc.sync.dma_start(ident_sb[:], ident[:])
        b1_sb = consts.tile([128, F // 128], f32, tag="b1", name="b1")
        nc.scalar.dma_start(b1_sb[:], b1t[:])
        onehot_sb = consts.tile([2, 128], bf16, tag="onehot", name="onehot")
        nc.gpsimd.dma_start(onehot_sb[:], onehot[:])
        eps_sb = consts.tile([128, 1], f32, tag="eps", name="eps")
        nc.vector.memset(eps_sb[:], EPS)

        def rep_const(ap2d, use, tag):
            if not use:
                return None
            t = consts.tile([128, D], f32, tag=tag, name=tag)
            nc.gpsimd.dma_start(out=t[:], in_=bcast_ap(ap2d, D))
            return t

        bo_rep = rep_const(bo, flags["use_bo"], "bo")
        b2_rep = rep_const(b2r, flags["use_b2"], "b2")
        g1_rep = rep_const(g1, flags["use_g1"], "g1")
        be1_rep = rep_const(be1, flags["use_be1"], "be1")
        g2_rep = rep_const(g2, flags["use_g2"], "g2")
        be2_rep = rep_const(be2, flags["use_be2"], "be2")

        # hT persists from the w_o phase into the FFN (top-level pool)
        hT = [poolP1.tile([128, RQ], bf16, tag=f"hT{k}", name=f"hT{k}")
              for k in range(KT)]
        # preloaded epilogue weights (loads run during attention)
        wop = [poolP1.tile([128, 2 * D], f8, tag=f"wo{j}", name=f"wo{j}")
               for j in range(KP)]
        w1a = [poolP1.tile([128, F // 2], bf16, tag=f"w1a{k}",
                           name=f"w1a{k}") for k in range(KT)]
        # ctxT pairs (fp8, DoubleRow stationary for w_o)
        ctxTp = [poolCtx.tile([128, 2 * RQ], f8, tag=f"ctxT{j}",
                              name=f"ctxT{j}") for j in range(KP)]

        # attention-lived arrays
        kTr = [poolP2.tile([128, L], bf16, tag=f"kTr{m}", name=f"kTr{m}")
               for m in range(KT)]
        qTr = [poolP2.tile([128, RQ], bf16, tag=f"qTr{m}", name=f"qTr{m}")
               for m in range(KT)]
        # v pairs (fp8, DoubleRow stationary for ctx); halves at 0/VPAD
        vpl = [poolP2.tile([128, 2 * VPAD], f8, tag=f"vl{i}", name=f"vl{i}")
               for i in range(KT)]
        vph = [poolP2.tile([128, 2 * VPAD], f8, tag=f"vh{i}", name=f"vh{i}")
               for i in range(KT)]
        ctxraw = [poolP2.tile([VSTR, RQ], bf16, tag=f"cr{h}", name=f"cr{h}")
                  for h in range(H)]

        # projection inputs (fp8 pairs for DoubleRow)
        xtp = [poolP3.tile([128, 2 * L], f8, tag=f"xt{j}", name=f"xt{j}")
               for j in range(KP)]
        xqp = [poolP3.tile([128, 2 * RQ], f8, tag=f"xq{j}", name=f"xq{j}")
               for j in range(KP)]
        wkp = [poolP3.tile([128, 2 * D], f8, tag=f"wk{j}", name=f"wk{j}")
               for j in range(KP)]
        wqp = [poolP3.tile([128, 2 * D], f8, tag=f"wq{j}", name=f"wq{j}")
               for j in range(KP)]
        wvp = [poolP3.tile([128, 2 * D], f8, tag=f"wv{j}", name=f"wv{j}")
               for j in range(KP)]
        cos_sb = poolP3.tile([128, L], bf16, tag="cos", name="cos")
        sin_sb = poolP3.tile([128, L], bf16, tag="sin", name="sin")
        qcos_sb = poolP3.tile([128, RQ], bf16, tag="qcos", name="qcos")
        qsin_sb = poolP3.tile([128, RQ], bf16, tag="qsin", name="qsin")

        def ld_pair(dsts, src, width):
            for j in range(KP):
                ld(dsts[j][:, 0:width], src[(2 * j) * 128:(2 * j + 1) * 128, :])
                ld(dsts[j][:, width:2 * width],
                   src[(2 * j + 1) * 128:(2 * j + 2) * 128, :])

        # load order: k-proj inputs first (they gate the exp stream)
        ld(cos_sb[:], cosr[:])
        ld(sin_sb[:], sinr[:])
        ld(qcos_sb[:], qcos[:])
        ld(qsin_sb[:], qsin[:])
        ld_pair(xtp, xT, L)
        ld_pair(wkp, wk, D)
        ld_pair(wqp, wq, D)
        ld_pair(xqp, xTq, RQ)
        ld_pair(wvp, wv, D)
        for j in range(KP):
            ld(wop[j][:, 0:D], wo[(2 * j) * 128:(2 * j + 1) * 128, :])
            ld(wop[j][:, D:2 * D],
               wo[(2 * j + 1) * 128:(2 * j + 2) * 128, :])
        for g in range(4):
            gsl = slice(g * 512, (g + 1) * 512)
            for k in range(KT):
                ld(w1a[k][:, gsl], w1[k * 128:(k + 1) * 128, gsl])

        xtv = [pairv(t) for t in xtp]
        xqv = [pairv(t) for t in xqp]
        wkv = [pairv(t) for t in wkp]
        wqv = [pairv(t) for t in wqp]
        wvv = [pairv(t) for t in wvp]

        def rope_chunk(ps, cos_sl, sin_sl, dst):
            """dst = ps*cos + shuffle16(ps*sin); tables carry the 1/64."""
            tct = poolRW.tile([128, 512], bf16, tag="rtc", name="rtc")
            nc.vector.tensor_mul(tct[:], ps, cos_sl)
            tsn = poolRW.tile([128, 512], bf16, tag="rtm", name="rtm")
            nc.vector.tensor_mul(tsn[:], ps, sin_sl)
            tsw = poolRW.tile([128, 512], bf16, tag="tsw", name="tsw")
            nc.vector.stream_shuffle(tsw[:], tsn[:],
                                     [(i + 16) % 32 for i in range(32)])
            nc.vector.tensor_add(dst, tct[:], tsw[:])

        with tc.tile_pool(name="pjps", bufs=2, space="PSUM") as pjps, \
             tc.tile_pool(name="scps", bufs=2, space="PSUM") as scps, \
             tc.tile_pool(name="ctxps", bufs=1, space="PSUM") as ctxps:

            def q_proj(m):
                msl = slice(m * 128, m * 128 + 128)
                ps = pjps.tile([128, 512], f32, tag="pj", name="pj")
                for j in range(KP):
                    nc.tensor.matmul(ps[:], wqv[j][:, :, msl], xqv[j][:],
                                     start=(j == 0), stop=(j == KP - 1),
                                     perf_mode=DR)
                rope_chunk(ps[:], qcos_sb[:], qsin_sb[:], qTr[m][:])

            def k_proj(m, n):
                msl = slice(m * 128, m * 128 + 128)
                nsl = slice(n * 512, n * 512 + 512)
                ps = pjps.tile([128, 512], f32, tag="pj", name="pj")
                for j in range(KP):
                    nc.tensor.matmul(ps[:], wkv[j][:, :, msl],
                                     xtv[j][:, :, nsl],
                                     start=(j == 0), stop=(j == KP - 1),
                                     perf_mode=DR)
                rope_chunk(ps[:], cos_sb[:, nsl], sin_sb[:, nsl],
                           kTr[m][:, nsl])

            # kTr[0] first (gates the exp stream), then q projection
            for n in range(NL):
                k_proj(0, n)
            for m in range(KT):
                q_proj(m)

            def v_proj(half, t):
                tsl = slice(t * 128, t * 128 + 128)
                ps = pjps.tile([128, 512], f32, tag="pj", name="pj")
                for j in range(KP):
                    nc.tensor.matmul(ps[:], xtv[j][:, :, tsl],
                                     wvv[j][:, :, half * 512:half * 512 + 512],
                                     start=(j == 0), stop=(j == KP - 1),
                                     perf_mode=DR)
                vp = (vpl if half == 0 else vph)[t // 2]
                off = (t % 2) * VPAD
                vview = vp[:, off:off + 8 * VSTR].rearrange(
                    "p (h e) -> p h e", h=8)
                ps_view = ps[:].rearrange("p (h e) -> p h e", h=8)
                nc.vector.tensor_copy(vview[:, :, 0:DK], ps_view[:])
                # ones column carries the 1/32 compensation for wv's x32
                nc.vector.memset(vview[:, :, DK:DK + 1], 32.0)

            # deferred projection work, emitted interleaved into attention
            stream = []
            for t in range(LT):
                stream.append(("v0", 0, t))
            for m in range(1, 5):
                for n in range(NL):
                    stream.append(("k", m, n))
            for t in range(LT):
                stream.append(("v1", 1, t))
            for m in range(5, KT):
                for n in range(NL):
                    stream.append(("k", m, n))
            v_pos = {}
            k_pos = {}
            for i, it in enumerate(stream):
                if it[0] == "k":
                    k_pos[it[1]] = i
                else:
                    v_pos[(it[1], it[2])] = i
            fed = [0]

            def emit(it):
                if it[0] == "k":
                    k_proj(it[1], it[2])
                else:
                    v_proj(it[1], it[2])

            def feed(n):
                for _ in range(n):
                    if fed[0] < len(stream):
                        emit(stream[fed[0]])
                        fed[0] += 1

            def drain_to(idx):
                while fed[0] <= idx:
                    emit(stream[fed[0]])
                    fed[0] += 1

            # ---- attention (projection stream fed between steps) ----
            for hp in range(KT):
                if hp >= 1:
                    drain_to(k_pos[hp])
                hA, hB = 2 * hp, 2 * hp + 1
                half = 0 if hp < 4 else 1
                vt = vpl if hp < 4 else vph
                ca = (hA % 8) * VSTR
                cb = (hB % 8) * VSTR
                cpsA = ctxps.tile([VSTR, RQ], f32, tag="cpsA", name="cpsA")
                cpsB = ctxps.tile([VSTR, RQ], f32, tag="cpsB", name="cpsB")

                def emit_ctx(ep, i):
                    epv = pairv(ep)
                    vv = pairv(vt[i])
                    nc.tensor.matmul(cpsA[:], vv[:, :, ca:ca + VSTR],
                                     epv[:, :, 0:RQ],
                                     start=(i == 0), stop=(i == KT - 1),
                                     perf_mode=DR)
                    nc.tensor.matmul(cpsB[:], vv[:, :, cb:cb + VSTR],
                                     epv[:, :, RQ:2 * RQ],
                                     start=(i == 0), stop=(i == KT - 1),
                                     perf_mode=DR)

                pend = None
                ep = None
                for kt in range(LT):
                    off = kt * 128
                    sc = scps.tile([128, 2 * RQ], f32, tag="sc", name="sc")
                    nc.tensor.matmul(sc[:, 0:RQ],
                                     kTr[hp][0:64, off:off + 128],
                                     qTr[hp][0:64, :], start=True, stop=True)
                    nc.tensor.matmul(sc[:, RQ:2 * RQ],
                                     kTr[hp][64:128, off:off + 128],
                                     qTr[hp][64:128, :], start=True, stop=True)
                    if kt % 2 == 0:
                        ep = poolE.tile([128, 4 * RQ], f8, tag="e", name="e")
                    nc.scalar.activation(
                        ep[:, (kt % 2) * 2 * RQ:(kt % 2 + 1) * 2 * RQ],
                        sc[:], AF.Exp, scale=0.125)
                    feed(1)
                    if kt % 2 == 1:
                        if pend is not None:
                            emit_ctx(*pend)
                        pend = (ep, kt // 2)
                    drain_to(v_pos[(half, min(kt + 1, LT - 1))])
                emit_ctx(*pend)
                nc.vector.tensor_copy(ctxraw[hA][:], cpsA[:])
                nc.vector.tensor_copy(ctxraw[hB][:], cpsB[:])
                s2 = poolCR.tile([2, RQ], bf16, tag="s2", name="s2")
                nc.sync.dma_start(s2[0:1, :], ctxraw[hA][64:65, :])
                nc.sync.dma_start(s2[1:2, :], ctxraw[hB][64:65, :])
                rec2 = poolCR.tile([2, RQ], f32, tag="rc", name="rc")
                nc.vector.reciprocal(rec2[:], s2[:])
                rcb = poolCR.tile([2, RQ], bf16, tag="rcb", name="rcb")
                nc.vector.tensor_copy(rcb[:], rec2[:])
                for h, cr in ((hA, ctxraw[hA]), (hB, ctxraw[hB])):
                    half2 = h % 2
                    rp = pjps.tile([128, 512], f32, tag="pj", name="pj")
                    nc.tensor.matmul(
                        rp[0:64, :],
                        onehot_sb[0:2, half2 * 64:half2 * 64 + 64],
                        rcb[:], start=True, stop=True)
                    dst = ctxTp[hp // 2][half2 * 64:half2 * 64 + 64,
                                         (hp % 2) * RQ:(hp % 2) * RQ + RQ]
                    nc.vector.tensor_mul(dst, cr[0:64, :], rp[0:64, :])

            stackP3.close()

        stackP2.close()

        # layer norm helper (takes its workspace pool)
        def layer_norm(dst, src, g_rep, be_rep, wpool):
            sview = src.rearrange("p (s d) -> p s d", s=2)
            stats = wpool.tile([128, 2, 6], f32, tag="lnstats",
                               name="lnstats")
            for sg in range(2):
                nc.vector.bn_stats(stats[:, sg, :], sview[:, sg, :])
            mv = wpool.tile([128, 2], f32, tag="lnmv", name="lnmv")
            nc.vector.bn_aggr(mv[:], stats[:])
            std = wpool.tile([128, 1], f32, tag="lnstd", name="lnstd")
            nc.scalar.activation(std[:], mv[:, 1:2], AF.Sqrt, bias=eps_sb[:])
            rstd = wpool.tile([128, 1], f32, tag="lnrstd", name="lnrstd")
            nc.vector.reciprocal(rstd[:], std[:])
            nc.vector.tensor_scalar(dst, src, mv[:, 0:1], rstd[:],
                                    op0=ALU.subtract, op1=ALU.mult)
            if g_rep is not None:
                nc.vector.tensor_mul(dst, dst, g_rep[:])
            if be_rep is not None:
                nc.vector.tensor_add(dst, dst, be_rep[:])

        # ---- w_o (fp8 DoubleRow) + residual + LN1 + transpose ----
        with tc.tile_pool(name="pwo", bufs=1) as poolWO, \
             tc.tile_pool(name="ph3w", bufs=2) as ph3w, \
             tc.tile_pool(name="aops", bufs=2, space="PSUM") as aops, \
             tc.tile_pool(name="tpps", bufs=4, space="PSUM") as tpps:
            xr_sb = [poolWO.tile([128, D], f32, tag=f"xr{t}", name=f"xr{t}")
                     for t in range(4)]
            for t in range(4):
                ld2(xr_sb[t][:], xr[t * 128:(t + 1) * 128, :])
            wov = [pairv(t) for t in wop]
            ctv = [pairv(t) for t in ctxTp]
            for qt in range(4):
                qsl = slice(qt * 128, qt * 128 + 128)
                ps = aops.tile([128, D], f32, tag="ao", name="ao")
                for half in range(2):
                    osl = slice(half * 512, half * 512 + 512)
                    for j in range(KP):
                        nc.tensor.matmul(ps[:, osl], ctv[j][:, :, qsl],
                                         wov[j][:, :, osl],
                                         start=(j == 0), stop=(j == KP - 1),
                                         perf_mode=DR)
                res = ph3w.tile([128, D], f32, tag="res", name="res")
                # fold the 1/64 w_o scale into the residual add
                nc.vector.scalar_tensor_tensor(
                    res[:], ps[:], 1.0 / 64.0, xr_sb[qt][:],
                    op0=ALU.mult, op1=ALU.add)
                if bo_rep is not None:
                    nc.vector.tensor_add(res[:], res[:], bo_rep[:])
                hq = ph3w.tile([128, D], f32, tag="hq", name="hq")
                layer_norm(hq[:], res[:], g1_rep, be1_rep, ph3w)
                nc.sync.dma_start(h_dram[qsl, :], hq[:])
                for m in range(KT):
                    tp = tpps.tile([128, 128], f32, tag="tp", name="tp")
                    nc.tensor.transpose(tp[:], hq[:, m * 128:m * 128 + 128],
                                        ident_sb[:])
                    nc.vector.tensor_copy(hT[m][:, qsl], tp[:])

        stackCtx.close()

        # ---- FFN (bf16; weights into all the freed space) ----
        with tc.tile_pool(name="pffn", bufs=1) as pffn, \
             tc.tile_pool(name="ph4w", bufs=2) as ph4w:
            ff1rT = [pffn.tile([128, RQ], bf16, tag=f"ff1{t}",
                               name=f"ff1{t}") for t in range(FT)]
            w1b = [pffn.tile([128, F // 2], bf16, tag=f"w1b{k}",
                             name=f"w1b{k}") for k in range(KT)]
            w2_sb = [pffn.tile([128, D], bf16, tag=f"w2{k}", name=f"w2{k}")
                     for k in range(FT)]
            # second w1 column half (first half preloaded during attention)
            for g in range(4):
                gsl = slice(g * 512, (g + 1) * 512)
                for k in range(KT):
                    ld2(w1b[k][:, gsl],
                        w1[k * 128:(k + 1) * 128, F // 2 + g * 512:
                           F // 2 + (g + 1) * 512])
            # w2 half-column-major
            for half in range(2):
                osl = slice(half * 512, half * 512 + 512)
                for k in range(FT):
                    ld2(w2_sb[k][:, osl], w2[k * 128:(k + 1) * 128, osl])

            # ---- FFN up + ReLU ----
            with tc.tile_pool(name="f1ps", bufs=4, space="PSUM") as f1ps:
                for ft in range(FT):
                    w1t = w1a if ft < FT // 2 else w1b
                    fo = (ft % (FT // 2)) * 128
                    ps = f1ps.tile([128, RQ], f32, tag="f1", name="f1")
                    for k in range(KT):
                        nc.tensor.matmul(ps[:], w1t[k][:, fo:fo + 128],
                                         hT[k][:],
                                         start=(k == 0), stop=(k == KT - 1))
                    nc.scalar.activation(ff1rT[ft][:], ps[:], AF.Relu,
                                         bias=b1_sb[:, ft:ft + 1])

            # ---- FFN down + LN2 ----
            with tc.tile_pool(name="f2ps", bufs=3, space="PSUM") as f2ps:
                for qt in range(4):
                    qsl = slice(qt * 128, qt * 128 + 128)
                    ps = f2ps.tile([128, D], f32, tag="f2", name="f2")
                    for half in range(2):
                        osl = slice(half * 512, half * 512 + 512)
                        for ft in range(FT):
                            nc.tensor.matmul(ps[:, osl], ff1rT[ft][:, qsl],
                                             w2_sb[ft][:, osl],
                                             start=(ft == 0),
                                             stop=(ft == FT - 1))
                    hback = ph4w.tile([128, D], f32, tag="hback",
                                      name="hback")
                    nc.sync.dma_start(hback[:], h_dram[qsl, :])
                    res = ph4w.tile([128, D], f32, tag="res2", name="res2")
                    nc.vector.tensor_add(res[:], ps[:], hback[:])
                    if b2_rep is not None:
                        nc.vector.tensor_add(res[:], res[:], b2_rep[:])
                    o = ph4w.tile([128, D], f32, tag="out", name="out")
                    layer_norm(o[:], res[:], g2_rep, be2_rep, ph4w)
                    nc.sync.dma_start(y[qt * 128:(qt + 1) * 128, :], o[:])

    return nc


_CACHED = {}


def _get_program(flags):
    key = tuple(sorted(flags.items()))
    if key not in _CACHED:
        _CACHED[key] = _build_program(flags)
    return _CACHED[key]


def kernel(x, w_q, w_k, w_v, w_o, b_o, gamma1, beta1, gamma2, beta2,
           w1, b1, w2, b2, _trace=False):
    _install_patches()
    from concourse import bass_utils

    bf = ml_dtypes.bfloat16
    f8h = ml_dtypes.float8_e4m3
    x = np.asarray(x, np.float32)
    flags = {
        "use_bo": not np.all(np.asarray(b_o) == 0),
        "use_b2": not np.all(np.asarray(b2) == 0),
        "use_g1": not np.all(np.asarray(gamma1) == 1),
        "use_be1": not np.all(np.asarray(beta1) == 0),
        "use_g2": not np.all(np.asarray(gamma2) == 1),
        "use_be2": not np.all(np.asarray(beta2) == 0),
    }
    nc = _get_program(flags)

    # host-side shared prep. Head dims are permuted so the RoPE partner
    # (d <-> d+32) sits 16 partitions away within the same 32-partition
    # quadrant, making the partner swap a DVE stream_shuffle:
    #   new position p (within a 64-dim head) holds old dim PI[p].
    PI = np.concatenate([np.arange(0, 16), np.arange(32, 48),
                         np.arange(16, 32), np.arange(48, 64)])
    inv_freq = (1.0 / (10000.0 ** (np.arange(0, DK, 2, dtype=np.float64) / DK)))
    freqs = np.arange(L, dtype=np.float64)[:, None] * inv_freq      # [L, 32]
    cos = np.cos(freqs).T.astype(np.float32)                        # [32, L]
    sin = np.sin(freqs).T.astype(np.float32)
    fidx = PI % 32
    sgn = np.where(PI < 32, 1.0, -1.0).astype(np.float32)[:, None]
    # tables carry the 1/64 compensation for the x64 fp8 weight scale
    cos_rep = (np.concatenate([cos[fidx], cos[fidx]], 0) / 64.0).astype(bf)
    sin_sign = (np.concatenate([sgn * sin[fidx], sgn * sin[fidx]], 0)
                / 64.0).astype(bf)

    def permute_heads(w):
        wp = np.asarray(w, np.float32).reshape(D, H, DK)
        return np.ascontiguousarray(wp[:, :, PI].reshape(D, D))

    common = {
        "cosr": cos_rep, "sinr": sin_sign,
        "wq": (permute_heads(w_q) * 64).astype(f8h),
        "wk": (permute_heads(w_k) * 64).astype(f8h),
        "wv": (np.asarray(w_v, np.float32) * 32).astype(f8h),
        "wo": (np.asarray(w_o, np.float32) * 64).astype(f8h),
        "w1": w1.astype(bf), "w2": w2.astype(bf),
        "b1t": np.ascontiguousarray(
            np.asarray(b1, np.float32).reshape(F // 128, 128).T),
        "ident": np.eye(128, dtype=np.float32),
        "onehot": np.concatenate(
            [np.concatenate([np.ones((1, 64), np.float32),
                             np.zeros((1, 64), np.float32)], 1),
             np.concatenate([np.zeros((1, 64), np.float32),
                             np.ones((1, 64), np.float32)], 1)],
            0).astype(bf),
        "bo": np.asarray(b_o, np.float32).reshape(1, D),
        "b2r": np.asarray(b2, np.float32).reshape(1, D),
        "g1": np.asarray(gamma1, np.float32).reshape(1, D),
        "be1": np.asarray(beta1, np.float32).reshape(1, D),
        "g2": np.asarray(gamma2, np.float32).reshape(1, D),
        "be2": np.asarray(beta2, np.float32).reshape(1, D),
    }
    xT_all = [np.ascontiguousarray(x[b].T).astype(f8h) for b in range(B)]

    in_maps = []
    for c in range(NCORES):
        b, r = c // 4, c % 4
        rows = slice(r * RQ, (r + 1) * RQ)
        m = dict(common)
        m["xT"] = xT_all[b]
        m["xTq"] = np.ascontiguousarray(xT_all[b][:, rows])
        m["xr"] = np.ascontiguousarray(x[b, rows, :])
        m["qcos"] = np.ascontiguousarray(cos_rep[:, rows])
        m["qsin"] = np.ascontiguousarray(sin_sign[:, rows])
        in_maps.append(m)

    res = bass_utils.run_bass_kernel_spmd(
        nc, in_maps, core_ids=list(range(NCORES)), trace=_trace)

    out = np.empty((B, L, D), np.float32)
    for c in range(NCORES):
        b, r = c // 4, c % 4
        out[b, r * RQ:(r + 1) * RQ, :] = res.results[c]["y"]
    if _trace:
        kernel.last_exec_time_ns = res.exec_time_ns
    return out


# revision 18
# speedup vs baseline: 1.2239x; 1.0018x over previous
"""Trainium2 Bass kernel for a dense transformer encoder layer.

Shapes (hardcoded): B=2, L=2048, D=1024, F=4096, H=16 heads, dk=64.
Sharding over 8 NeuronCores: core c handles batch b=c//4 and query-row
quarter r=c%4 (512 rows). K/V projections for the full batch are
computed per core (replicated within the 4-core batch group; collectives
measured too slow here due to cross-core launch skew).

Performance structure:
- Projections are fused INTO the attention loop: the softmax exp stream
  (scalar engine) is attention's long pole, and K/V projection matmuls
  are emitted interleaved with score/context matmuls.
- Q/K/V/O projections and the attention context matmul run in fp8
  (e4m3) with DoubleRow perf mode (2 contraction tiles per matmul,
  ~1.4x tensor throughput). Weights are pre-scaled by 64 (w_v by 32)
  to stay in fp8 normal range; the scale is compensated for free in the
  RoPE tables (/64), the softmax ones-column (=32), and the w_o
  residual add (x 1/64). The FFN stays bf16 (fp8 there costs too much
  accuracy).
- RoPE's partner swap is a DVE stream_shuffle: head dims are permuted
  host-side so partners sit 16 partitions apart within one quadrant.
- Rescale is fused per head-pair; the epilogue (w_o + LN1 + FFN + LN2)
  is pipelined per query tile.
"""
import os
import sys
import types

sys.path.insert(0, "/opt/trn_rl_repo")

import numpy as np
import ml_dtypes

import concourse.bass as bass
import concourse.tile as tile
import concourse.mybir as mybir
from contextlib import ExitStack

f32 = mybir.dt.float32
bf16 = mybir.dt.bfloat16
f8 = mybir.dt.float8e4
AF = mybir.ActivationFunctionType
ALU = mybir.AluOpType
DR = mybir.MatmulPerfMode.DoubleRow

B, L, D, F, H, DK = 2, 2048, 1024, 4096, 16, 64
RQ = 512          # query rows per core
NCORES = 8
EPS = 1e-6
KT = D // 128      # 8 contraction tiles over D
KP = KT // 2       # 4 DoubleRow contraction pairs
NL = L // 512      # 4 column chunks over L
LT = L // 128      # 16 key tiles
FT = F // 128      # 32 f-tiles
VSTR = 65          # per-head stride in v tiles (64 v cols + ones)
VPAD = 528         # fp8 pair stride for v tiles (16-aligned)

_PATCHED = False


def _install_patches():
    """Register the NTFF profile hook (if available) and wrap the BIR
    compile step to split multi-wait instructions (this walrus build
    accepts at most one sync-wait per instruction)."""
    global _PATCHED
    if _PATCHED:
        return
    _PATCHED = True

    if "antenv.axon_hooks" not in sys.modules:
        try:
            from trn_agent_boot.trn_boot import _ntff_profile_via_ctypes
            hook = _ntff_profile_via_ctypes("/opt/axon/libaxon_pjrt.so")
        except Exception:
            hook = None
        mod = types.ModuleType("antenv.axon_hooks")
        mod.get_axon_ntff_profile_hook = lambda: hook
        mod.set_axon_ntff_profile_hook = lambda h: None
        sys.modules["antenv.axon_hooks"] = mod

    import json

    def _split_multiwaits(bir_bytes):
        d = json.loads(bir_bytes)
        ctr = 0
        for fn in d.get("functions", []):
            for blk in fn.get("blocks", []):
                out = []
                for inst in blk.get("instructions", []):
                    si = inst.get("sync_info")
                    ow = (si or {}).get("on_wait") or []
                    if len(ow) > 1 and inst.get("engine", "Unassigned") != "Unassigned":
                        for w in ow[:-1]:
                            out.append({
                                "debug": inst.get("debug", 0),
                                "engine": inst["engine"],
                                "ins": [], "outs": [],
                                "name": f"I-antsw{ctr}",
                                "opcode": "NoOp",
                                "sync_info": {"on_update": [], "on_wait": [w]},
                            })
                            ctr += 1
                        si["on_wait"] = [ow[-1]]
                    out.append(inst)
                blk["instructions"] = out
        return json.dumps(d).encode()

    import concourse.bass_utils as bu
    import concourse.bass2jax as b2j

    orig = bu.compile_bir_kernel

    def patched(bir_json, tmpdir, neff_name="file.neff"):
        return orig(_split_multiwaits(bir_json), tmpdir, neff_name=neff_name)

    bu.compile_bir_kernel = patched
    b2j.compile_bir_kernel = patched


def _build_program(flags):
    nc = bass.Bass("TRN2", target_bir_lowering=False, debug=False,
                   num_devices=NCORES)

    def din(name, shape, dt):
        return nc.dram_tensor(name, shape, dt, kind="ExternalInput").ap()

    xT = din("xT", [D, L], f8)              # batch x, transposed, fp8
    xTq = din("xTq", [D, RQ], f8)           # this core's columns of x[b].T
    xr = din("xr", [RQ, D], f32)            # this core's rows (residual)
    cosr = din("cosr", [128, L], bf16)      # cos table (permuted, /64)
    sinr = din("sinr", [128, L], bf16)      # sign-baked sin table (/64)
    qcos = din("qcos", [128, RQ], bf16)
    qsin = din("qsin", [128, RQ], bf16)
    wq = din("wq", [D, D], f8)              # x64, head-dims permuted
    wk = din("wk", [D, D], f8)              # x64, head-dims permuted
    wv = din("wv", [D, D], f8)              # x32
    wo = din("wo", [D, D], f8)              # x64
    w1 = din("w1", [D, F], bf16)
    w2 = din("w2", [F, D], bf16)
    b1t = din("b1t", [128, F // 128], f32)
    ident = din("ident", [128, 128], f32)
    onehot = din("onehot", [2, 128], bf16)
    bo = din("bo", [1, D], f32)
    b2r = din("b2r", [1, D], f32)
    g1 = din("g1", [1, D], f32)
    be1 = din("be1", [1, D], f32)
    g2 = din("g2", [1, D], f32)
    be2 = din("be2", [1, D], f32)
    y = nc.dram_tensor("y", [RQ, D], f32, kind="ExternalOutput").ap()

    def bcast_ap(ap2d, width):
        return bass.AP(tensor=ap2d.tensor, offset=ap2d.offset,
                       ap=[[0, 128], [1, width]])

    def pairv(t):
        # [128, 2*W] tile -> [128, 2, W] DoubleRow view
        return t[:].rearrange("p (a b) -> p a b", a=2)

    with tile.TileContext(nc) as tc:
      with ExitStack() as top:
        consts = top.enter_context(tc.tile_pool(name="consts", bufs=1))
        poolP1 = top.enter_context(tc.tile_pool(name="p1", bufs=1))
        dramp = top.enter_context(tc.tile_pool(name="dramp", bufs=1,
                                               space="DRAM"))
        h_dram = dramp.tile([RQ, D], f32, tag="h_dram", name="h_dram")
        stackCtx = ExitStack()
        poolCtx = stackCtx.enter_context(tc.tile_pool(name="pctx", bufs=1))
        stackP2 = ExitStack()
        poolP2 = stackP2.enter_context(tc.tile_pool(name="p2", bufs=1))
        poolE = stackP2.enter_context(tc.tile_pool(name="pe", bufs=3))
        poolCR = stackP2.enter_context(tc.tile_pool(name="pcr", bufs=2))
        stackP3 = ExitStack()
        poolP3 = stackP3.enter_context(tc.tile_pool(name="p3", bufs=1))
        poolRW = stackP3.enter_context(tc.tile_pool(name="prw", bufs=2))

        _qs = [nc.sync, nc.scalar, nc.gpsimd]
        _qi = [0]

        def ld(dst, src):
            _qs[_qi[0] % 3].dma_start(dst, src)
            _qi[0] += 1

        _qs2 = [nc.sync, nc.gpsimd]
        _qi2 = [0]

        def ld2(dst, src):
            # loads issued while the exp stream owns the scalar queue
            _qs2[_qi2[0] % 2].dma_start(dst, src)
            _qi2[0] += 1

        # --- long-lived constants ---
        ident_sb = consts.tile([128, 128], f32, tag="ident", name="ident")
        nc.sync.dma_start(ident_sb[:], ident[:])
        b1_sb = consts.tile([128, F // 128], f32, tag="b1", name="b1")
        nc.scalar.dma_start(b1_sb[:], b1t[:])
        onehot_sb = consts.tile([2, 128], bf16, tag="onehot", name="onehot")
        nc.gpsimd.dma_start(onehot_sb[:], onehot[:])
        eps_sb = consts.tile([128, 1], f32, tag="eps", name="eps")
        nc.vector.memset(eps_sb[:], EPS)

        def rep_const(ap2d, use, tag):
            if not use:
                return None
            t = consts.tile([128, D], f32, tag=tag, name=tag)
            nc.gpsimd.dma_start(out=t[:], in_=bcast_ap(ap2d, D))
            return t

        bo_rep = rep_const(bo, flags["use_bo"], "bo")
        b2_rep = rep_const(b2r, flags["use_b2"], "b2")
        g1_rep = rep_const(g1, flags["use_g1"], "g1")
        be1_rep = rep_const(be1, flags["use_be1"], "be1")
        g2_rep = rep_const(g2, flags["use_g2"], "g2")
        be2_rep = rep_const(be2, flags["use_be2"], "be2")

        # hT persists from the w_o phase into the FFN (top-level pool)
        hT = [poolP1.tile([128, RQ], bf16, tag=f"hT{k}", name=f"hT{k}")
              for k in range(KT)]
        # preloaded epilogue weights (loads run during attention)
        wop = [poolP1.tile([128, 2 * D], f8, tag=f"wo{j}", name=f"wo{j}")
               for j in range(KP)]
        w1a = [poolP1.tile([128, F // 2], bf16, tag=f"w1a{k}",
                           name=f"w1a{k}") for k in range(KT)]
        # ctxT pairs (fp8, DoubleRow stationary for w_o)
        ctxTp = [poolCtx.tile([128, 2 * RQ], f8, tag=f"ctxT{j}",
                              name=f"ctxT{j}") for j in range(KP)]

        # attention-lived arrays
        kTr = [poolP2.tile([128, L], bf16, tag=f"kTr{m}", name=f"kTr{m}")
               for m in range(KT)]
        qTr = [poolP2.tile([128, RQ], bf16, tag=f"qTr{m}", name=f"qTr{m}")
               for m in range(KT)]
        # v pairs (fp8, DoubleRow stationary for ctx); halves at 0/VPAD
        vpl = [poolP2.tile([128, 2 * VPAD], f8, tag=f"vl{i}", name=f"vl{i}")
               for i in range(KT)]
        vph = [poolP2.tile([128, 2 * VPAD], f8, tag=f"vh{i}", name=f"vh{i}")
               for i in range(KT)]
        ctxraw = [poolP2.tile([VSTR, RQ], bf16, tag=f"cr{h}", name=f"cr{h}")
                  for h in range(H)]

        # projection inputs (fp8 pairs for DoubleRow)
        xtp = [poolP3.tile([128, 2 * L], f8, tag=f"xt{j}", name=f"xt{j}")
               for j in range(KP)]
        xqp = [poolP3.tile([128, 2 * RQ], f8, tag=f"xq{j}", name=f"xq{j}")
               for j in range(KP)]
        wkp = [poolP3.tile([128, 2 * D], f8, tag=f"wk{j}", name=f"wk{j}")
               for j in range(KP)]
        wqp = [poolP3.tile([128, 2 * D], f8, tag=f"wq{j}", name=f"wq{j}")
               for j in range(KP)]
        wvp = [poolP3.tile([128, 2 * D], f8, tag=f"wv{j}", name=f"wv{j}")
               for j in range(KP)]
        cos_sb = poolP3.tile([128, L], bf16, tag="cos", name="cos")
        sin_sb = poolP3.tile([128, L], bf16, tag="sin", name="sin")
        qcos_sb = poolP3.tile([128, RQ], bf16, tag="qcos", name="qcos")
        qsin_sb = poolP3.tile([128, RQ], bf16, tag="qsin", name="qsin")

        def ld_pair(dsts, src, width):
            for j in range(KP):
                ld(dsts[j][:, 0:width], src[(2 * j) * 128:(2 * j + 1) * 128, :])
                ld(dsts[j][:, width:2 * width],
                   src[(2 * j + 1) * 128:(2 * j + 2) * 128, :])

        # load order: k-proj inputs first (they gate the exp stream)
        ld(cos_sb[:], cosr[:])
        ld(sin_sb[:], sinr[:])
        ld(qcos_sb[:], qcos[:])
        ld(qsin_sb[:], qsin[:])
        ld_pair(xtp, xT, L)
        ld_pair(wkp, wk, D)
        ld_pair(wqp, wq, D)
        ld_pair(xqp, xTq, RQ)
        ld_pair(wvp, wv, D)
        for j in range(KP):
            ld(wop[j][:, 0:D], wo[(2 * j) * 128:(2 * j + 1) * 128, :])
            ld(wop[j][:, D:2 * D],
               wo[(2 * j + 1) * 128:(2 * j + 2) * 128, :])
        for g in range(4):
            gsl = slice(g * 512, (g + 1) * 512)
            for k in range(KT):
                ld(w1a[k][:, gsl], w1[k * 128:(k + 1) * 128, gsl])

        xtv = [pairv(t) for t in xtp]
        xqv = [pairv(t) for t in xqp]
        wkv = [pairv(t) for t in wkp]
        wqv = [pairv(t) for t in wqp]
        wvv = [pairv(t) for t in wvp]

        def rope_chunk(ps, cos_sl, sin_sl, dst):
            """dst = ps*cos + shuffle16(ps*sin); tables carry the 1/64.
            The PSUM evacuation runs on the scalar engine (it has slack
            between exps) so the projection PSUM slot recycles fast."""
            pb = poolRW.tile([128, 512], bf16, tag="rpb", name="rpb")
            nc.scalar.copy(pb[:], ps)
            tct = poolRW.tile([128, 512], bf16, tag="rtc", name="rtc")
            nc.vector.tensor_mul(tct[:], pb[:], cos_sl)
            tsn = poolRW.tile([128, 512], bf16, tag="rtm", name="rtm")
            nc.vector.tensor_mul(tsn[:], pb[:], sin_sl)
            tsw = poolRW.tile([128, 512], bf16, tag="tsw", name="tsw")
            nc.vector.stream_shuffle(tsw[:], tsn[:],
                                     [(i + 16) % 32 for i in range(32)])
            nc.vector.tensor_add(dst, tct[:], tsw[:])

        with tc.tile_pool(name="pjps", bufs=2, space="PSUM") as pjps, \
             tc.tile_pool(name="scps", bufs=2, space="PSUM") as scps, \
             tc.tile_pool(name="ctxps", bufs=1, space="PSUM") as ctxps:

            def q_proj(m):
                msl = slice(m * 128, m * 128 + 128)
                ps = pjps.tile([128, 512], f32, tag="pj", name="pj")
                for j in range(KP):
                    nc.tensor.matmul(ps[:], wqv[j][:, :, msl], xqv[j][:],
                                     start=(j == 0), stop=(j == KP - 1),
                                     perf_mode=DR)
                rope_chunk(ps[:], qcos_sb[:], qsin_sb[:], qTr[m][:])

            def k_proj(m, n):
                msl = slice(m * 128, m * 128 + 128)
                nsl = slice(n * 512, n * 512 + 512)
                ps = pjps.tile([128, 512], f32, tag="pj", name="pj")
                for j in range(KP):
                    nc.tensor.matmul(ps[:], wkv[j][:, :, msl],
                                     xtv[j][:, :, nsl],
                                     start=(j == 0), stop=(j == KP - 1),
                                     perf_mode=DR)
                rope_chunk(ps[:], cos_sb[:, nsl], sin_sb[:, nsl],
                           kTr[m][:, nsl])

            # kTr[0] first (gates the exp stream), then q projection
            for n in range(NL):
                k_proj(0, n)
            for m in range(KT):
                q_proj(m)

            def v_proj(half, t):
                tsl = slice(t * 128, t * 128 + 128)
                ps = pjps.tile([128, 512], f32, tag="pj", name="pj")
                for j in range(KP):
                    nc.tensor.matmul(ps[:], xtv[j][:, :, tsl],
                                     wvv[j][:, :, half * 512:half * 512 + 512],
                                     start=(j == 0), stop=(j == KP - 1),
                                     perf_mode=DR)
                vp = (vpl if half == 0 else vph)[t // 2]
                off = (t % 2) * VPAD
                vview = vp[:, off:off + 8 * VSTR].rearrange(
                    "p (h e) -> p h e", h=8)
                ps_view = ps[:].rearrange("p (h e) -> p h e", h=8)
                nc.vector.tensor_copy(vview[:, :, 0:DK], ps_view[:])
                # ones column carries the 1/32 compensation for wv's x32
                nc.vector.memset(vview[:, :, DK:DK + 1], 32.0)

            # deferred projection work, emitted interleaved into attention
            stream = []
            for t in range(LT):
                stream.append(("v0", 0, t))
            for m in range(1, 5):
                for n in range(NL):
                    stream.append(("k", m, n))
            for t in range(LT):
                stream.append(("v1", 1, t))
            for m in range(5, KT):
                for n in range(NL):
                    stream.append(("k", m, n))
            v_pos = {}
            k_pos = {}
            for i, it in enumerate(stream):
                if it[0] == "k":
                    k_pos[it[1]] = i
                else:
                    v_pos[(it[1], it[2])] = i
            fed = [0]

            def emit(it):
                if it[0] == "k":
                    k_proj(it[1], it[2])
                else:
                    v_proj(it[1], it[2])

            def feed(n):
                for _ in range(n):
                    if fed[0] < len(stream):
                        emit(stream[fed[0]])
                        fed[0] += 1

            def drain_to(idx):
                while fed[0] <= idx:
                    emit(stream[fed[0]])
                    fed[0] += 1

            # ---- attention (projection stream fed between steps) ----
            for hp in range(KT):
                if hp >= 1:
                    drain_to(k_pos[hp])
                hA, hB = 2 * hp, 2 * hp + 1
                half = 0 if hp < 4 else 1
                vt = vpl if hp < 4 else vph
                ca = (hA % 8) * VSTR
                cb = (hB % 8) * VSTR
                cpsA = ctxps.tile([VSTR, RQ], f32, tag="cpsA", name="cpsA")
                cpsB = ctxps.tile([VSTR, RQ], f32, tag="cpsB", name="cpsB")

                def emit_ctx(ep, i):
                    epv = pairv(ep)
                    vv = pairv(vt[i])
                    nc.tensor.matmul(cpsA[:], vv[:, :, ca:ca + VSTR],
                                     epv[:, :, 0:RQ],
                                     start=(i == 0), stop=(i == KT - 1),
                                     perf_mode=DR)
                    nc.tensor.matmul(cpsB[:], vv[:, :, cb:cb + VSTR],
                                     epv[:, :, RQ:2 * RQ],
                                     start=(i == 0), stop=(i == KT - 1),
                                     perf_mode=DR)

                pend = None
                ep = None
                for kt in range(LT):
                    off = kt * 128
                    sc = scps.tile([128, 2 * RQ], f32, tag="sc", name="sc")
                    nc.tensor.matmul(sc[:, 0:RQ],
                                     kTr[hp][0:64, off:off + 128],
                                     qTr[hp][0:64, :], start=True, stop=True)
                    nc.tensor.matmul(sc[:, RQ:2 * RQ],
                                     kTr[hp][64:128, off:off + 128],
                                     qTr[hp][64:128, :], start=True, stop=True)
                    if kt % 2 == 0:
                        ep = poolE.tile([128, 4 * RQ], f8, tag="e", name="e")
                    nc.scalar.activation(
                        ep[:, (kt % 2) * 2 * RQ:(kt % 2 + 1) * 2 * RQ],
                        sc[:], AF.Exp, scale=0.125)
                    if kt < LT - 3:
                        feed(1)
                    if kt % 2 == 1:
                        if pend is not None:
                            emit_ctx(*pend)
                        pend = (ep, kt // 2)
                    drain_to(v_pos[(half, min(kt + 1, LT - 1))])
                emit_ctx(*pend)
                nc.vector.tensor_copy(ctxraw[hA][:], cpsA[:])
                nc.vector.tensor_copy(ctxraw[hB][:], cpsB[:])
                s2 = poolCR.tile([2, RQ], bf16, tag="s2", name="s2")
                nc.sync.dma_start(s2[0:1, :], ctxraw[hA][64:65, :])
                nc.sync.dma_start(s2[1:2, :], ctxraw[hB][64:65, :])
                rec2 = poolCR.tile([2, RQ], f32, tag="rc", name="rc")
                nc.vector.reciprocal(rec2[:], s2[:])
                rcb = poolCR.tile([2, RQ], bf16, tag="rcb", name="rcb")
                nc.vector.tensor_copy(rcb[:], rec2[:])
                for h, cr in ((hA, ctxraw[hA]), (hB, ctxraw[hB])):
                    half2 = h % 2
                    rp = pjps.tile([128, 512], f32, tag="pj", name="pj")
                    nc.tensor.matmul(
                        rp[0:64, :],
                        onehot_sb[0:2, half2 * 64:half2 * 64 + 64],
                        rcb[:], start=True, stop=True)
                    dst = ctxTp[hp // 2][half2 * 64:half2 * 64 + 64,
                                         (hp % 2) * RQ:(hp % 2) * RQ + RQ]
                    nc.vector.tensor_mul(dst, cr[0:64, :], rp[0:64, :])
                feed(3)

            # switch the ACT table set to sqrt while the last rescale
            # trails, so LN1's first sqrt doesn't pay the ~2.7us load
            dsq = poolCR.tile([2, RQ], f32, tag="dsq", name="dsq")
            nc.scalar.activation(dsq[0:1, 0:1], eps_sb[0:1, 0:1], AF.Sqrt,
                                 bias=eps_sb[0:1, :])

            stackP3.close()

        stackP2.close()

        # layer norm helper (takes its workspace pool)
        def layer_norm(dst, src, g_rep, be_rep, wpool):
            sview = src.rearrange("p (s d) -> p s d", s=2)
            stats = wpool.tile([128, 2, 6], f32, tag="lnstats",
                               name="lnstats")
            for sg in range(2):
                nc.vector.bn_stats(stats[:, sg, :], sview[:, sg, :])
            mv = wpool.tile([128, 2], f32, tag="lnmv", name="lnmv")
            nc.vector.bn_aggr(mv[:], stats[:])
            std = wpool.tile([128, 1], f32, tag="lnstd", name="lnstd")
            nc.scalar.activation(std[:], mv[:, 1:2], AF.Sqrt, bias=eps_sb[:])
            rstd = wpool.tile([128, 1], f32, tag="lnrstd", name="lnrstd")
            nc.vector.reciprocal(rstd[:], std[:])
            nc.vector.tensor_scalar(dst, src, mv[:, 0:1], rstd[:],
                                    op0=ALU.subtract, op1=ALU.mult)
            if g_rep is not None:
                nc.vector.tensor_mul(dst, dst, g_rep[:])
            if be_rep is not None:
                nc.vector.tensor_add(dst, dst, be_rep[:])

        # ---- w_o (fp8 DoubleRow) + residual + LN1 + transpose ----
        with tc.tile_pool(name="pwo", bufs=1) as poolWO, \
             tc.tile_pool(name="ph3w", bufs=2) as ph3w, \
             tc.tile_pool(name="aops", bufs=2, space="PSUM") as aops, \
             tc.tile_pool(name="tpps", bufs=4, space="PSUM") as tpps:
            xr_sb = [poolWO.tile([128, D], f32, tag=f"xr{t}", name=f"xr{t}")
                     for t in range(4)]
            for t in range(4):
                ld2(xr_sb[t][:], xr[t * 128:(t + 1) * 128, :])
            wov = [pairv(t) for t in wop]
            ctv = [pairv(t) for t in ctxTp]
            for qt in range(4):
                qsl = slice(qt * 128, qt * 128 + 128)
                ps = aops.tile([128, D], f32, tag="ao", name="ao")
                for half in range(2):
                    osl = slice(half * 512, half * 512 + 512)
                    for j in range(KP):
                        nc.tensor.matmul(ps[:, osl], ctv[j][:, :, qsl],
                                         wov[j][:, :, osl],
                                         start=(j == 0), stop=(j == KP - 1),
                                         perf_mode=DR)
                res = ph3w.tile([128, D], f32, tag="res", name="res")
                # fold the 1/64 w_o scale into the residual add
                nc.vector.scalar_tensor_tensor(
                    res[:], ps[:], 1.0 / 64.0, xr_sb[qt][:],
                    op0=ALU.mult, op1=ALU.add)
                if bo_rep is not None:
                    nc.vector.tensor_add(res[:], res[:], bo_rep[:])
                hq = ph3w.tile([128, D], f32, tag="hq", name="hq")
                layer_norm(hq[:], res[:], g1_rep, be1_rep, ph3w)
                nc.sync.dma_start(h_dram[qsl, :], hq[:])
                for m in range(KT):
                    tp = tpps.tile([128, 128], f32, tag="tp", name="tp")
                    nc.tensor.transpose(tp[:], hq[:, m * 128:m * 128 + 128],
                                        ident_sb[:])
                    nc.vector.tensor_copy(hT[m][:, qsl], tp[:])

        stackCtx.close()

        # ---- FFN (bf16; weights into all the freed space) ----
        with tc.tile_pool(name="pffn", bufs=1) as pffn, \
             tc.tile_pool(name="ph4w", bufs=2) as ph4w:
            ff1rT = [pffn.tile([128, RQ], bf16, tag=f"ff1{t}",
                               name=f"ff1{t}") for t in range(FT)]
            w1b = [pffn.tile([128, F // 2], bf16, tag=f"w1b{k}",
                             name=f"w1b{k}") for k in range(KT)]
            w2_sb = [pffn.tile([128, D], bf16, tag=f"w2{k}", name=f"w2{k}")
                     for k in range(FT)]
            # second w1 column half (first half preloaded during attention)
            for g in range(4):
                gsl = slice(g * 512, (g + 1) * 512)
                for k in range(KT):
                    ld2(w1b[k][:, gsl],
                        w1[k * 128:(k + 1) * 128, F // 2 + g * 512:
                           F // 2 + (g + 1) * 512])
            # w2 half-column-major
            for half in range(2):
                osl = slice(half * 512, half * 512 + 512)
                for k in range(FT):
                    ld2(w2_sb[k][:, osl], w2[k * 128:(k + 1) * 128, osl])

            # ---- FFN up + ReLU ----
            with tc.tile_pool(name="f1ps", bufs=4, space="PSUM") as f1ps:
                for ft in range(FT):
                    w1t = w1a if ft < FT // 2 else w1b
                    fo = (ft % (FT // 2)) * 128
                    ps = f1ps.tile([128, RQ], f32, tag="f1", name="f1")
                    for k in range(KT):
                        nc.tensor.matmul(ps[:], w1t[k][:, fo:fo + 128],
                                         hT[k][:],
                                         start=(k == 0), stop=(k == KT - 1))
                    nc.scalar.activation(ff1rT[ft][:], ps[:], AF.Relu,
                                         bias=b1_sb[:, ft:ft + 1])

            # ---- FFN down + LN2 ----
            with tc.tile_pool(name="f2ps", bufs=3, space="PSUM") as f2ps:
                for qt in range(4):
                    qsl = slice(qt * 128, qt * 128 + 128)
                    ps = f2ps.tile([128, D], f32, tag="f2", name="f2")
                    for half in range(2):
                        osl = slice(half * 512, half * 512 + 512)
                        for ft in range(FT):
                            nc.tensor.matmul(ps[:, osl], ff1rT[ft][:, qsl],
                                             w2_sb[ft][:, osl],
                                             start=(ft == 0),
                                             stop=(ft == FT - 1))
                    hback = ph4w.tile([128, D], f32, tag="hback",
                                      name="hback")
                    nc.sync.dma_start(hback[:], h_dram[qsl, :])
                    res = ph4w.tile([128, D], f32, tag="res2", name="res2")
                    nc.vector.tensor_add(res[:], ps[:], hback[:])
                    if b2_rep is not None:
                        nc.vector.tensor_add(res[:], res[:], b2_rep[:])
                    o = ph4w.tile([128, D], f32, tag="out", name="out")
                    layer_norm(o[:], res[:], g2_rep, be2_rep, ph4w)
                    nc.sync.dma_start(y[qt * 128:(qt + 1) * 128, :], o[:])

    return nc


_CACHED = {}


def _get_program(flags):
    key = tuple(sorted(flags.items()))
    if key not in _CACHED:
        _CACHED[key] = _build_program(flags)
    return _CACHED[key]


def kernel(x, w_q, w_k, w_v, w_o, b_o, gamma1, beta1, gamma2, beta2,
           w1, b1, w2, b2, _trace=False):
    _install_patches()
    from concourse import bass_utils

    bf = ml_dtypes.bfloat16
    f8h = ml_dtypes.float8_e4m3
    x = np.asarray(x, np.float32)
    flags = {
        "use_bo": not np.all(np.asarray(b_o) == 0),
        "use_b2": not np.all(np.asarray(b2) == 0),
        "use_g1": not np.all(np.asarray(gamma1) == 1),
        "use_be1": not np.all(np.asarray(beta1) == 0),
        "use_g2": not np.all(np.asarray(gamma2) == 1),
        "use_be2": not np.all(np.asarray(beta2) == 0),
    }
    nc = _get_program(flags)

    # host-side shared prep. Head dims are permuted so the RoPE partner
    # (d <-> d+32) sits 16 partitions away within the same 32-partition
    # quadrant, making the partner swap a DVE stream_shuffle:
    #   new position p (within a 64-dim head) holds old dim PI[p].
    PI = np.concatenate([np.arange(0, 16), np.arange(32, 48),
                         np.arange(16, 32), np.arange(48, 64)])
    inv_freq = (1.0 / (10000.0 ** (np.arange(0, DK, 2, dtype=np.float64) / DK)))
    freqs = np.arange(L, dtype=np.float64)[:, None] * inv_freq      # [L, 32]
    cos = np.cos(freqs).T.astype(np.float32)                        # [32, L]
    sin = np.sin(freqs).T.astype(np.float32)
    fidx = PI % 32
    sgn = np.where(PI < 32, 1.0, -1.0).astype(np.float32)[:, None]
    # tables carry the 1/64 compensation for the x64 fp8 weight scale
    cos_rep = (np.concatenate([cos[fidx], cos[fidx]], 0) / 64.0).astype(bf)
    sin_sign = (np.concatenate([sgn * sin[fidx], sgn * sin[fidx]], 0)
                / 64.0).astype(bf)

    def permute_heads(w):
        wp = np.asarray(w, np.float32).reshape(D, H, DK)
        return np.ascontiguousarray(wp[:, :, PI].reshape(D, D))

    common = {
        "cosr": cos_rep, "sinr": sin_sign,
        "wq": (permute_heads(w_q) * 64).astype(f8h),
        "wk": (permute_heads(w_k) * 64).astype(f8h),
        "wv": (np.asarray(w_v, np.float32) * 32).astype(f8h),
        "wo": (np.asarray(w_o, np.float32) * 64).astype(f8h),
        "w1": w1.astype(bf), "w2": w2.astype(bf),
        "b1t": np.ascontiguousarray(
            np.asarray(b1, np.float32).reshape(F // 128, 128).T),
        "ident": np.eye(128, dtype=np.float32),
        "onehot": np.concatenate(
            [np.concatenate([np.ones((1, 64), np.float32),
                             np.zeros((1, 64), np.float32)], 1),
             np.concatenate([np.zeros((1, 64), np.float32),
                             np.ones((1, 64), np.float32)], 1)],
            0).astype(bf),
        "bo": np.asarray(b_o, np.float32).reshape(1, D),
        "b2r": np.asarray(b2, np.float32).reshape(1, D),
        "g1": np.asarray(gamma1, np.float32).reshape(1, D),
        "be1": np.asarray(beta1, np.float32).reshape(1, D),
        "g2": np.asarray(gamma2, np.float32).reshape(1, D),
        "be2": np.asarray(beta2, np.float32).reshape(1, D),
    }
    xT_all = [np.ascontiguousarray(x[b].T).astype(f8h) for b in range(B)]

    in_maps = []
    for c in range(NCORES):
        b, r = c // 4, c % 4
        rows = slice(r * RQ, (r + 1) * RQ)
        m = dict(common)
        m["xT"] = xT_all[b]
        m["xTq"] = np.ascontiguousarray(xT_all[b][:, rows])
        m["xr"] = np.ascontiguousarray(x[b, rows, :])
        m["qcos"] = np.ascontiguousarray(cos_rep[:, rows])
        m["qsin"] = np.ascontiguousarray(sin_sign[:, rows])
        in_maps.append(m)

    res = bass_utils.run_bass_kernel_spmd(
        nc, in_maps, core_ids=list(range(NCORES)), trace=_trace)

    out = np.empty((B, L, D), np.float32)
    for c in range(NCORES):
        b, r = c // 4, c % 4
        out[b, r * RQ:(r + 1) * RQ, :] = res.results[c]["y"]
    if _trace:
        kernel.last_exec_time_ns = res.exec_time_ns
    return out
